# revision 1
# baseline (speedup 1.0000x reference)
"""Trainium2 Bass kernel for nn_MultiCrossAttention (PVT-style multi-scale
spatial-reduction cross attention).

Sharding: data-parallel over batch (B=32 -> 4 per core x 8 cores), weights
replicated.  All heavy matmuls run in float32r (TF32-like) at 1 cycle/row.

Per-batch pipeline (all "T" tensors are channel-major [c, n]):
  y_i --(contig h-band DMA)--> w-pool tree (DVE/GPSIMD) -> fused
  h-pool+transpose matmuls (PE, pool matrix Ah) -> poolT [c,256] ->
  1x1 conv matmuls (+bias via K=1 matmul) -> channel-major LN (colsum
  matmuls for stats, rank-1 outer-product matmuls for broadcast) -> GELU
  -> xcT.  x -> PE transpose -> xT -> q matmuls -> qT; LN+GELU(xT) -> x4T.
  kv matmuls -> kT (chan-major) + v (token-major, with ones column for
  softmax denominators).  Scores computed TRANSPOSED (sT[nk,nq]) so softmax
  denominator comes free out of the PV matmul's 65th row; normalization via
  reciprocal + rank-1 broadcast.  proj matmuls -> out.
"""

import sys

sys.path.insert(0, "/opt/trn_rl_repo")

import numpy as np

import concourse.bass as bass
import concourse.mybir as mybir
import concourse.tile as tile
from concourse.bass_utils import run_bass_kernel_spmd
from concourse.masks import make_identity

# ---------------------------------------------------------------------------
# Patch: this walrus build only accepts ONE sync-wait on a Drain instruction;
# Tile's tail drain waits on every live semaphore lane.  Split it into a chain
# of single-wait drains.
from concourse.vector_clock import ScopedClock, VectorClock
from concourse.tile_sem_assignment import N_PROCS


def _patched_drain_and_barrier(self, tick_clock, wait_clock):
    nc = self.nc
    gc = tick_clock.global_clock
    nz = [p for p in range(N_PROCS) if gc[p] > 0]
    groups = [nz[i : i + 1] for i in range(0, len(nz), 1)] or [[]]
    for g in groups[:-1]:
        masked = VectorClock([gc[p] if p in g else 0 for p in range(N_PROCS)])
        d = nc.sync.drain()
        wait_clock.add_sem_waits(d.ins, ScopedClock({None: masked}))
    drain_inst = nc.sync.drain()
    last = ScopedClock(
        {None: VectorClock([gc[p] if p in groups[-1] else 0 for p in range(N_PROCS)])}
    )
    wait_clock.add_sem_waits(drain_inst.ins, last)
    nc.all_engine_barrier()
    assert self.sems is not None
    popped = nc._tile_sem_poison_stack.pop()
    assert popped is self._sem_poison
    nc.clear_and_free_semaphores(list(self.sems.allocated().values()))
    nc.all_engine_barrier()


tile.TileContext._drain_and_barrier = _patched_drain_and_barrier


def _split_excess_waits(nc, limit=1):
    """Walrus in this build rejects >2 sync-waits on compute/DMA instructions
    (>1 on Drain).  Move excess waits onto same-engine no-ops inserted just
    before the offending instruction."""
    import bass_rust

    uid = [0]
    for f in nc.m.functions:
        for blk in f.blocks:
            newlist = []
            changed = False
            for ins in blk.instructions:
                si = ins.sync_info
                waits = list(si.on_wait) if si and si.on_wait else []
                tn = type(ins).__name__
                lim = 1 if tn in ("InstDrain", "InstNoOp", "InstTensorTensor") else limit
                if len(waits) > lim:
                    keep = waits[-lim:]
                    for w in waits[:-lim]:
                        nop = bass_rust.InstNoOp(
                            name=f"wsplit-{uid[0]}", ins=[], outs=[]
                        )
                        uid[0] += 1
                        nop.engine = ins.engine
                        nop.sync_info = mybir.SyncInfo(on_wait=[w], on_update=[])
                        newlist.append(nop)
                    ins.sync_info = mybir.SyncInfo(
                        on_wait=keep,
                        on_update=list(si.on_update) if si.on_update else [],
                    )
                    changed = True
                newlist.append(ins)
            if changed:
                blk.instructions = newlist


# ---------------------------------------------------------------------------

F32 = mybir.dt.float32
F32R = mybir.dt.float32r
AF = mybir.ActivationFunctionType

NCORES = 8
B = 32
BPC = B // NCORES  # batches per core
N1 = 256  # query tokens
C1 = 512
NH, HD = 8, 64
SCALE = HD ** -0.5
EPS = 1e-5
C2 = (64, 128, 320)
RATIO = (8, 4, 2)
HW = (128, 64, 32)  # spatial side per branch
GRP = (1, 2, 4)  # w-groups packed into partitions (128 = H*G)
NKV = 256  # kv tokens (16x16 pooled grid for every branch)

# xc channel-permutation: kt bins of 128 rows; each branch ptile lands at a
# 64-aligned partition base.  Global xc order: x1 0:64 | x2 64:192 | x3
# 192:512 | x4 512:1024.
# kt0=[x1 | x3c], kt1=x2, kt2=x3a, kt3=x3b, kt4..7=x4
_PERM = np.concatenate(
    [
        np.arange(0, 64),  # x1        -> kt0[0:64]
        np.arange(448, 512),  # x3 pt2  -> kt0[64:128]
        np.arange(64, 192),  # x2       -> kt1
        np.arange(192, 320),  # x3 pt0  -> kt2
        np.arange(320, 448),  # x3 pt1  -> kt3
        np.arange(512, 1024),  # x4     -> kt4..7
    ]
)
# (kt, base) of each branch ptile in xcT
XC_SLOT = {
    "y1": [(0, 0)],
    "y2": [(1, 0)],
    "y3": [(2, 0), (3, 0), (0, 64)],
    "x4": [(4, 0), (5, 0), (6, 0), (7, 0)],
}


def _pool_mats():
    """Ah matrices: [128, G*16] mapping partition (h,g) -> col (g*16+ho),
    with the full 1/r^2 divisor folded in."""
    out = []
    for i in range(3):
        G, r = GRP[i], RATIO[i]
        H = 128 // G
        m = np.zeros((128, G * 16), dtype=np.float32)
        for h in range(H):
            for g in range(G):
                p = h * G + g
                ho = h // r
                m[p, g * 16 + ho] = 1.0 / (r * r)
        out.append(m)
    return out


def build_module(debug=False, reps=1):
    nc = bass.Bass(trn_type="TRN2")
    dbg_d = {}
    if debug:
        for nm, shp in [
            ("d_poolt1", [64, NKV]), ("d_poolt2", [128, NKV]),
            ("d_poolt3", [128, 3, NKV]), ("d_xT", [128, 4, NKV]),
            ("d_qT", [128, 4, NKV]), ("d_xcT", [128, 8, NKV]),
            ("d_kT", [128, 4, NKV]), ("d_vaug", [128, 2, NH, HD + 1]),
            ("d_ste0", [128, 2, NKV]), ("d_outT", [128, 4, NKV]),
        ]:
            dbg_d[nm] = nc.dram_tensor(nm, shp, F32, kind="ExternalOutput")

    # ---- DRAM I/O -------------------------------------------------------
    x_d = nc.dram_tensor("x", [BPC, N1, C1], F32, kind="ExternalInput")
    y1_d = nc.dram_tensor("y1", [BPC, 128 * 128, 64], F32, kind="ExternalInput")
    y2_d = nc.dram_tensor("y2", [BPC, 64 * 64, 128], F32, kind="ExternalInput")
    y3_d = nc.dram_tensor("y3", [BPC, 32 * 32, 320], F32, kind="ExternalInput")
    wq_d = nc.dram_tensor("wq_t", [C1, C1], F32, kind="ExternalInput")
    wkv_d = nc.dram_tensor("wkv_t", [1024, 1024], F32, kind="ExternalInput")
    proj_d = nc.dram_tensor("proj_t", [C1, C1], F32, kind="ExternalInput")
    projb_d = nc.dram_tensor("projb", [C1], F32, kind="ExternalInput")
    srw_d = [
        nc.dram_tensor(f"srw{i+1}_t", [C2[i], C2[i]], F32, kind="ExternalInput")
        for i in range(3)
    ]
    srb_d = [
        nc.dram_tensor(f"srb{i+1}", [C2[i]], F32, kind="ExternalInput")
        for i in range(3)
    ]
    ah_d = [
        nc.dram_tensor(f"ah{i+1}", [128, GRP[i] * 16], F32, kind="ExternalInput")
        for i in range(3)
    ]
    g_d = [
        nc.dram_tensor(f"g{i+1}", [C2[i]], F32, kind="ExternalInput") for i in range(3)
    ] + [nc.dram_tensor("g4", [C1], F32, kind="ExternalInput")]
    ng_d = [
        nc.dram_tensor(f"ng{i+1}", [C2[i]], F32, kind="ExternalInput")
        for i in range(3)
    ] + [nc.dram_tensor("ng4", [C1], F32, kind="ExternalInput")]
    b_d = [
        nc.dram_tensor(f"lb{i+1}", [C2[i]], F32, kind="ExternalInput")
        for i in range(3)
    ] + [nc.dram_tensor("lb4", [C1], F32, kind="ExternalInput")]
    out_d = nc.dram_tensor("out", [BPC, N1, C1], F32, kind="ExternalOutput")

    CB = [64, 128, 320, 512]  # channels per branch (incl. x4)
    NPT = [1, 1, 3, 4]  # partition tiles per branch

    with tile.TileContext(nc) as tc:
        with (
            tc.tile_pool(name="wts", bufs=1) as wts,
            tc.tile_pool(name="bands", bufs=2) as bandp,
            tc.tile_pool(name="t1", bufs=1) as t1p,
            tc.tile_pool(name="poolt", bufs=1) as pooltp,
            tc.tile_pool(name="bbuf", bufs=1) as bbufp,
            tc.tile_pool(name="work", bufs=1) as work,
            tc.tile_pool(name="rows", bufs=4) as rowsp,
            tc.tile_pool(name="sq", bufs=2) as sqp,
            tc.tile_pool(name="ste", bufs=1) as step,
            tc.tile_pool(name="outb", bufs=1) as outbp,
            tc.tile_pool(name="pp1", bufs=1, space="PSUM") as pp1,
        ):
            # ---- load weights (one-time; gpsimd DMA casts f32 -> f32r) ----
            wq_s = wts.tile([128, 4, C1], F32R)
            nc.gpsimd.dma_start(
                out=wq_s, in_=wq_d.ap().rearrange("(t p) o -> p t o", p=128)
            )
            wkv_s = wts.tile([128, 8, 1024], F32R)
            nc.gpsimd.dma_start(
                out=wkv_s, in_=wkv_d.ap().rearrange("(t p) o -> p t o", p=128)
            )
            proj_s = wts.tile([128, 4, C1], F32R)
            nc.gpsimd.dma_start(
                out=proj_s, in_=proj_d.ap().rearrange("(t p) o -> p t o", p=128)
            )
            projb_s = wts.tile([128, C1], F32)
            nc.scalar.dma_start(
                out=projb_s,
                in_=bass.AP(tensor=projb_d, offset=0, ap=[[0, 128], [1, C1]]),
            )
            srw_s = []
            for i in range(3):
                c = C2[i]
                nkt = (c + 127) // 128
                t = wts.tile([min(c, 128), nkt, c], F32R, tag=f"srw{i}", name=f"srw{i}")
                if c <= 128:
                    nc.gpsimd.dma_start(out=t[:, 0], in_=srw_d[i].ap())
                else:
                    full = (c // 128) * 128
                    nc.gpsimd.dma_start(
                        out=t[:, : c // 128],
                        in_=srw_d[i]
                        .ap()[0:full]
                        .rearrange("(t p) o -> p t o", p=128),
                    )
                    if c % 128:
                        nc.gpsimd.dma_start(
                            out=t[: c % 128, c // 128], in_=srw_d[i].ap()[full:c]
                        )
                srw_s.append(t)
            srb_s = [
                wts.tile([1, C2[i]], F32R, tag=f"srb{i}", name=f"srb{i}")
                for i in range(3)
            ]
            for i in range(3):
                nc.gpsimd.dma_start(
                    out=srb_s[i],
                    in_=bass.AP(tensor=srb_d[i], offset=0, ap=[[0, 1], [1, C2[i]]]),
                )
            ah_s = []
            for i in range(3):
                t = wts.tile([128, GRP[i] * 16], F32R, tag=f"ah{i}", name=f"ah{i}")
                nc.gpsimd.dma_start(out=t, in_=ah_d[i].ap())
                ah_s.append(t)
            g_s, ng_s, b_s = [], [], []
            for i in range(4):
                c = CB[i]
                gt = wts.tile([1, c], F32R, tag=f"g{i}", name=f"g{i}")
                ngt = wts.tile([1, c], F32R, tag=f"ng{i}", name=f"ng{i}")
                bt = wts.tile([1, c], F32R, tag=f"b{i}", name=f"b{i}")
                nc.gpsimd.dma_start(
                    out=gt, in_=bass.AP(tensor=g_d[i], offset=0, ap=[[0, 1], [1, c]])
                )
                nc.gpsimd.dma_start(
                    out=ngt, in_=bass.AP(tensor=ng_d[i], offset=0, ap=[[0, 1], [1, c]])
                )
                nc.gpsimd.dma_start(
                    out=bt, in_=bass.AP(tensor=b_d[i], offset=0, ap=[[0, 1], [1, c]])
                )
                g_s.append(gt)
                ng_s.append(ngt)
                b_s.append(bt)

            ident = wts.tile([128, 128], F32)
            make_identity(nc, ident)
            onescol = wts.tile([128, 1], F32R)
            nc.vector.memset(onescol.bitcast(F32), 1.0)
            onesrow = wts.tile([1, NKV], F32R)
            nc.vector.memset(onesrow.bitcast(F32), 1.0)
            ones64 = wts.tile([1, 64], F32R)
            nc.vector.memset(ones64.bitcast(F32), 1.0)
            epsrow = wts.tile([1, 1], F32)
            nc.gpsimd.memset(epsrow, EPS)

            y1r = y1_d.ap().rearrange("b (h w) c -> b h (w c)", h=128)
            y2r = y2_d.ap().rearrange("b (h wb wi) c -> b (h wb) (wi c)", wb=2, wi=32)
            y3r = y3_d.ap().rearrange("b (h wb wi) c -> b (h wb) (wi c)", wb=4, wi=8)
            xr = x_d.ap().rearrange("b (nt p) c -> b p nt c", p=128)
            outr = out_d.ap().rearrange("b (nt p) c -> b p nt c", p=128)

            for rep in range(reps):
             for bi in range(BPC):
                # ==== x: load + transpose -> xT; q matmuls ==============
                x_sb = work.tile([128, 2, C1], F32, tag="x_sb")
                nc.sync.dma_start(out=x_sb, in_=xr[bi])
                xT = work.tile([128, 4, NKV], F32R, tag="xT", bufs=2)
                for cc in range(4):
                    tp = pp1.tile([128, 2, 128], F32, tag="ppB", name="xtp", bufs=2)
                    for nt in range(2):
                        nc.tensor.transpose(
                            tp[:, nt], x_sb[:, nt, cc * 128 : (cc + 1) * 128], ident
                        )
                    nc.scalar.copy(out=xT[:, cc], in_=tp.rearrange("p a b -> p (a b)"))

                qT = work.tile([128, 4, NKV], F32R, tag="qT", bufs=2)
                for mt in range(4):
                    qp = pp1.tile([128, NKV], F32, tag="ppB", name="qp", bufs=2)
                    for kt in range(4):
                        nc.tensor.matmul(
                            qp,
                            wq_s[:, kt, mt * 128 : (mt + 1) * 128],
                            xT[:, kt],
                            start=(kt == 0),
                            stop=(kt == 3),
                        )
                    nc.scalar.copy(out=qT[:, mt], in_=qp)

                # ==== branch pooling ====================================
                # ---- y1: two halves, 8->1 w-tree (3 levels) ----
                t1y1 = t1p.tile([128, 16, 64], F32R, tag="t1y1")
                for qt in range(4):
                    band = bandp.tile([128, 2048], F32, tag="bandq", bufs=2)
                    nc.sync.dma_start(
                        out=band, in_=y1r[bi, :, qt * 2048 : (qt + 1) * 2048]
                    )
                    v = band.rearrange("p (wo dw c) -> p wo dw c", wo=4, dw=8)
                    nc.gpsimd.tensor_add(v[:, :, 0:4], v[:, :, 0:4], v[:, :, 4:8])
                    nc.vector.tensor_add(v[:, :, 0:2], v[:, :, 0:2], v[:, :, 2:4])
                    nc.vector.tensor_add(
                        t1y1[:, qt * 4 : (qt + 1) * 4], v[:, :, 0], v[:, :, 1]
                    )
                # ---- y2: two half-bands, 4->1 tree (2 levels) ----
                t1y2 = t1p.tile([128, 8, 128], F32R, tag="t1y2")
                for hf in range(2):
                    band = bandp.tile([128, 2048], F32, tag="band2", name="band2", bufs=2)
                    nc.sync.dma_start(
                        out=band, in_=y2r[bi, :, hf * 2048 : (hf + 1) * 2048]
                    )
                    v = band.rearrange("p (wo dw c) -> p wo dw c", wo=4, dw=4)
                    nc.gpsimd.tensor_add(v[:, :, 0:2], v[:, :, 0:2], v[:, :, 2:4])
                    nc.vector.tensor_add(
                        t1y2[:, hf * 4 : (hf + 1) * 4], v[:, :, 0], v[:, :, 1]
                    )
                # ---- y3: single band, 2->1 tree ----
                t1y3 = t1p.tile([128, 4, 320], F32R, tag="t1y3")
                band3 = bandp.tile([128, 2560], F32, tag="band3", bufs=1)
                v3f = band3.rearrange("p (wo dw c) -> p wo dw c", wo=4, dw=2)
                nc.sync.dma_start(out=band3, in_=y3r[bi])
                nc.gpsimd.tensor_add(t1y3, v3f[:, :, 0], v3f[:, :, 1])

                # ---- fused h-pool + transpose (PE) -> poolT ----
                poolp1 = pp1.tile([64, 16, 16], F32, tag="ppA", name="poolp1", bufs=3)
                for wo in range(16):
                    nc.tensor.matmul(
                        poolp1[:, wo], t1y1[:, wo], ah_s[0], start=True, stop=True
                    )
                poolt1 = pooltp.tile([64, NKV], F32R, tag="poolt1")
                nc.scalar.copy(out=poolt1, in_=poolp1.rearrange("c a b -> c (a b)"))

                poolp2 = pp1.tile([128, 2, 8, 16], F32, tag="ppA", name="poolp2", bufs=3)
                for wo in range(8):
                    nc.tensor.matmul(
                        poolp2[:, :, wo], t1y2[:, wo], ah_s[1], start=True, stop=True
                    )
                poolt2 = pooltp.tile([128, NKV], F32R, tag="poolt2")
                nc.scalar.copy(out=poolt2, in_=poolp2.rearrange("c g a b -> c (g a b)"))

                poolt3 = pooltp.tile([128, 3, NKV], F32R, tag="poolt3")
                for cs in range(3):
                    cl = 64 if cs == 2 else 128
                    poolp3 = pp1.tile([128, 4, 4, 16], F32, tag="ppA", name="poolp3", bufs=3)
                    for wo in range(4):
                        nc.tensor.matmul(
                            poolp3[:cl, :, wo],
                            t1y3[:, wo, cs * 128 : cs * 128 + cl],
                            ah_s[2],
                            start=True,
                            stop=True,
                        )
                    nc.scalar.copy(
                        out=poolt3[:cl, cs],
                        in_=poolp3[:cl].rearrange("c g a b -> c (g a b)"),
                    )

                # ==== branch conv + LN + GELU -> xcT ====================
                xcT = work.tile([128, 8, NKV], F32R, tag="xcT")
                poolts = [poolt1, poolt2, poolt3]

                for br in [3, 1, 2, 0]:
                    cb = CB[br]
                    npt = NPT[br]
                    # conv -> preP psum tiles (list per ptile), or x4: use xT
                    datas = []  # sbuf fp32r [cpt, 256] data tiles per ptile
                    if br < 3:
                        bb = bbufp.tile([128, npt, NKV], F32R, tag=f"bb{br}", name=f"bb{br}")
                        sqs = []
                        for pt in range(npt):
                            cl = min(128, cb - pt * 128)
                            prep = pp1.tile([128, NKV], F32, tag="ppA", name="prep", bufs=3)
                            nkt = (cb + 127) // 128
                            for kt in range(nkt):
                                kl = min(128, cb - kt * 128)
                                if br < 2:
                                    lhs = srw_s[br][
                                        :kl, 0, pt * 128 : pt * 128 + cl
                                    ]
                                    rhs = poolts[br][:kl]
                                else:
                                    lhs = srw_s[2][:kl, kt, pt * 128 : pt * 128 + cl]
                                    rhs = poolts[2][:kl, kt]
                                nc.tensor.matmul(
                                    prep[:cl], lhs, rhs, start=(kt == 0), stop=False
                                )
                            # bias via K=1 matmul with ones row
                            nc.tensor.matmul(
                                prep[:cl],
                                srb_s[br][:, pt * 128 : pt * 128 + cl],
                                onesrow,
                                start=False,
                                stop=True,
                            )
                            nc.scalar.copy(out=bb[:cl, pt], in_=prep[:cl])
                            sq = sqp.tile([128, NKV], F32R, tag="x4sq", name="bsq")
                            nc.scalar.activation(
                                out=sq[:cl], in_=prep[:cl], func=AF.Square
                            )
                            sqs.append(sq)
                            datas.append(bb[:cl, pt])
                    else:
                        x4sqs = []
                        for kt in range(4):
                            sq = sqp.tile([128, NKV], F32R, tag="x4sq")
                            nc.scalar.activation(
                                out=sq, in_=xT[:, kt], func=AF.Square
                            )
                            x4sqs.append(sq)
                            datas.append(xT[:, kt])
                        stat_rhs = None

                    # column sums (PE): accumulate over ptiles
                    stats = pp1.tile([1, 2, NKV], F32, tag="ppA", name="stats", bufs=3)
                    if br < 3:
                        for pt in range(npt):
                            cl = min(128, cb - pt * 128)
                            nc.tensor.matmul(
                                stats[:, 0],
                                onescol[:cl],
                                bb[:cl, pt],
                                start=(pt == 0),
                                stop=(pt == npt - 1),
                                skip_group_check=True,
                            )
                        for pt in range(npt):
                            cl = min(128, cb - pt * 128)
                            nc.tensor.matmul(
                                stats[:, 1],
                                onescol[:cl],
                                sqs[pt][:cl],
                                start=(pt == 0),
                                stop=(pt == npt - 1),
                                skip_group_check=True,
                            )
                    else:
                        for kt in range(4):
                            nc.tensor.matmul(
                                stats[:, 0],
                                onescol,
                                datas[kt],
                                start=(kt == 0),
                                stop=(kt == 3),
                                skip_group_check=True,
                            )
                        for kt in range(4):
                            nc.tensor.matmul(
                                stats[:, 1],
                                onescol,
                                x4sqs[kt],
                                start=(kt == 0),
                                stop=(kt == 3),
                                skip_group_check=True,
                            )

                    # stats -> mean / rstd rows
                    mrow = rowsp.tile([1, NKV], F32, tag="tmprow", name="mrow")
                    r1row = rowsp.tile([1, NKV], F32, tag="tmprow", name="r1row")
                    nc.vector.tensor_scalar_mul(mrow, stats[:, 0], 1.0 / cb)
                    nc.vector.tensor_scalar_mul(r1row, stats[:, 1], 1.0 / cb)
                    msq = rowsp.tile([1, NKV], F32, tag="tmprow", name="msq")
                    nc.scalar.activation(out=msq, in_=mrow, func=AF.Square)
                    var = rowsp.tile([1, NKV], F32, tag="tmprow", name="var")
                    nc.vector.tensor_sub(var, r1row, msq)
                    sd = rowsp.tile([1, NKV], F32, tag="tmprow", name="sd")
                    nc.scalar.activation(
                        out=sd, in_=var, func=AF.Sqrt, bias=epsrow
                    )
                    rstd_r = rowsp.tile([1, NKV], F32R, tag="rstd_r", bufs=2)
                    with nc.allow_low_precision(reason="tf32 rstd is fine"):
                        nc.vector.reciprocal(rstd_r, sd)
                    mr_r = rowsp.tile([1, NKV], F32R, tag="mr_r", bufs=2)
                    nc.vector.tensor_mul(mr_r, mrow, rstd_r)

                    # rank-1 broadcasts + normalize + gelu into xcT slots
                    for pt in range(NPT[br]):
                        cl = min(128, cb - pt * 128)
                        kt_slot, base = XC_SLOT[["y1", "y2", "y3", "x4"][br]][pt]
                        S = pp1.tile([128, NKV], F32, tag="ppA", name="Sbc", bufs=3)
                        nc.tensor.matmul(
                            S[:cl],
                            g_s[br][:, pt * 128 : pt * 128 + cl],
                            rstd_r,
                            start=True,
                            stop=True,
                        )
                        Bb = pp1.tile([128, NKV], F32, tag="ppA", name="Bbc", bufs=3)
                        nc.tensor.matmul(
                            Bb[:cl],
                            b_s[br][:, pt * 128 : pt * 128 + cl],
                            onesrow,
                            start=True,
                            stop=False,
                        )
                        nc.tensor.matmul(
                            Bb[:cl],
                            ng_s[br][:, pt * 128 : pt * 128 + cl],
                            mr_r,
                            start=False,
                            stop=True,
                        )
                        dst = xcT[base : base + cl, kt_slot]
                        if br < 3:
                            # branch data is already in tau=(wo*16+ho) order
                            nc.vector.tensor_mul(dst, datas[pt], S[:cl])
                            nc.vector.tensor_add(dst, dst, Bb[:cl])
                            nc.scalar.activation(out=dst, in_=dst, func=AF.Gelu)
                        else:
                            # x4 tokens are in natural (ho*16+wo) order; permute
                            # the gelu's write AP to tau so all kv channels of a
                            # token refer to the same spatial position.
                            tmpn = sqp.tile(
                                [128, NKV], F32R, tag="nrm", name="nrm"
                            )
                            nc.vector.tensor_mul(tmpn, datas[pt], S[:cl])
                            nc.vector.tensor_add(tmpn, tmpn, Bb[:cl])
                            nc.scalar.activation(
                                out=dst.rearrange(
                                    "c (wo ho) -> c ho wo", wo=16
                                ),
                                in_=tmpn.rearrange(
                                    "c (ho wo) -> c ho wo", ho=16
                                ),
                                func=AF.Gelu,
                            )

                # ==== kv matmuls ========================================
                KTORD = [4, 5, 6, 7, 1, 2, 3, 0]
                kT = work.tile([128, 4, NKV], F32R, tag="kT")
                for mt in range(4):
                    kp = pp1.tile([128, NKV], F32, tag="ppB", name="kp", bufs=2)
                    for i, kt in enumerate(KTORD):
                        nc.tensor.matmul(
                            kp,
                            wkv_s[:, kt, mt * 128 : (mt + 1) * 128],
                            xcT[:, kt],
                            start=(i == 0),
                            stop=(i == 7),
                        )
                    nc.scalar.copy(out=kT[:, mt], in_=kp)

                v_aug = work.tile([128, 2, NH, HD + 1], F32R, tag="v_aug", bufs=2)
                nc.vector.memset(v_aug[:, :, :, HD : HD + 1].bitcast(F32), 1.0)
                for mt in range(2):
                    vp = pp1.tile([128, C1], F32, tag="ppB", name="vp", bufs=2)
                    for i, kt in enumerate(KTORD):
                        nc.tensor.matmul(
                            vp,
                            xcT[:, kt, mt * 128 : (mt + 1) * 128],
                            wkv_s[:, kt, 512:1024],
                            start=(i == 0),
                            stop=(i == 7),
                        )
                    nc.scalar.copy(
                        out=v_aug[:, mt, :, 0:HD],
                        in_=vp.rearrange("p (h d) -> p h d", h=NH),
                    )

                # ==== attention per head ================================
                outT = work.tile([128, 4, NKV], F32R, tag="outT", bufs=2)
                for h in range(NH):
                    pb = (h % 2) * 64
                    ck = h // 2
                    sp = pp1.tile([128, 2, NKV], F32, tag="ppC", name="sp", bufs=2)
                    for nt in range(2):
                        nc.tensor.matmul(
                            sp[:, nt],
                            kT[pb : pb + 64, ck, nt * 128 : (nt + 1) * 128],
                            qT[pb : pb + 64, ck],
                            start=True,
                            stop=True,
                        )
                    ste = step.tile([128, 2, NKV], F32R, tag="ste")
                    nc.scalar.activation(out=ste, in_=sp, func=AF.Exp, scale=SCALE)
                    if debug and bi == 0 and h == 0:
                        nc.sync.dma_start(out=dbg_d["d_ste0"].ap(), in_=ste.bitcast(F32))
                    pv = pp1.tile([65, NKV], F32, tag="ppC", name="pv", bufs=2)
                    for nt in range(2):
                        nc.tensor.matmul(
                            pv,
                            v_aug[:, nt, h],
                            ste[:, nt],
                            start=(nt == 0),
                            stop=(nt == 1),
                        )
                    rs_r = rowsp.tile([1, NKV], F32R, tag="rs_r", bufs=2)
                    with nc.allow_low_precision(reason="tf32 softmax denom"):
                        nc.vector.reciprocal(rs_r, pv[64:65])
                    bc = pp1.tile([64, NKV], F32, tag="ppC", name="bc", bufs=2)
                    nc.tensor.matmul(bc, ones64, rs_r, start=True, stop=True)
                    bcs = step.tile([64, NKV], F32, tag="bcs")
                    nc.scalar.copy(out=bcs, in_=bc)
                    nc.vector.tensor_mul(outT[pb : pb + 64, ck], pv[0:64], bcs)

                # ==== proj + bias + store ===============================
                osb = outbp.tile([128, 2, C1], F32, tag="osb")
                for nt in range(2):
                    fp = pp1.tile([128, C1], F32, tag="ppD", name="fp", bufs=1)
                    for kt in range(4):
                        nc.tensor.matmul(
                            fp,
                            outT[:, kt, nt * 128 : (nt + 1) * 128],
                            proj_s[:, kt],
                            start=(kt == 0),
                            stop=(kt == 3),
                        )
                    nc.vector.tensor_add(osb[:, nt], fp, projb_s)
                nc.sync.dma_start(out=outr[bi], in_=osb)
                if debug and bi == 0:
                    for nm, tl in [
                        ("d_poolt1", poolt1), ("d_poolt2", poolt2),
                        ("d_poolt3", poolt3), ("d_xT", xT), ("d_qT", qT),
                        ("d_xcT", xcT), ("d_kT", kT), ("d_vaug", v_aug),
                        ("d_outT", outT),
                    ]:
                        nc.sync.dma_start(
                            out=dbg_d[nm].ap(), in_=tl.bitcast(F32)
                        )

    _split_excess_waits(nc)
    return nc


def kernel(**inputs):
    x = np.ascontiguousarray(inputs["x"], dtype=np.float32)
    y1 = np.ascontiguousarray(inputs["y1"], dtype=np.float32)
    y2 = np.ascontiguousarray(inputs["y2"], dtype=np.float32)
    y3 = np.ascontiguousarray(inputs["y3"], dtype=np.float32)
    Wq = np.asarray(inputs["Wq"], dtype=np.float32)
    Wkv = np.asarray(inputs["Wkv"], dtype=np.float32)
    proj_w = np.asarray(inputs["proj_w"], dtype=np.float32)
    proj_b = np.asarray(inputs["proj_b"], dtype=np.float32)

    wq_t = np.ascontiguousarray(Wq.T)
    wkv_t = np.ascontiguousarray(Wkv.T[_PERM, :])
    proj_t = np.ascontiguousarray(proj_w.T)
    ah = _pool_mats()

    common = {
        "wq_t": wq_t,
        "wkv_t": wkv_t,
        "proj_t": proj_t,
        "projb": proj_b,
        "ah1": ah[0],
        "ah2": ah[1],
        "ah3": ah[2],
    }
    for i in range(3):
        common[f"srw{i+1}_t"] = np.ascontiguousarray(
            np.asarray(inputs[f"sr{i+1}_w"], dtype=np.float32).T
        )
        common[f"srb{i+1}"] = np.asarray(inputs[f"sr{i+1}_b"], dtype=np.float32)
        g = np.asarray(inputs[f"ln{i+1}_g"], dtype=np.float32)
        common[f"g{i+1}"] = g
        common[f"ng{i+1}"] = -g
        common[f"lb{i+1}"] = np.asarray(inputs[f"ln{i+1}_b"], dtype=np.float32)
    g4 = np.asarray(inputs["ln4_g"], dtype=np.float32)
    common["g4"] = g4
    common["ng4"] = -g4
    common["lb4"] = np.asarray(inputs["ln4_b"], dtype=np.float32)

    nc = build_module()
    in_maps = []
    for c in range(NCORES):
        sl = slice(c * BPC, (c + 1) * BPC)
        m = dict(common)
        m["x"] = x[sl]
        m["y1"] = y1[sl]
        m["y2"] = y2[sl]
        m["y3"] = y3[sl]
        in_maps.append(m)

    res = run_bass_kernel_spmd(nc, in_maps, core_ids=list(range(NCORES)))
    return np.concatenate([r["out"] for r in res.results], axis=0)


if __name__ == "__main__":
    pass



# revision 44
# speedup vs baseline: 43195.7062x; 43195.7062x over previous
"""Trainium2 Bass kernel for nn_MultiCrossAttention (PVT-style multi-scale
spatial-reduction cross attention).

Sharding: data-parallel over batch (B=32 -> 4 per core x 8 cores), weights
replicated.

v2 design (vs baseline):
  * bf16 weights packed host-side (halves weight DMA + SBUF, guarantees
    1 cyc/row matmuls); activations copied out of PSUM as bf16.
  * LN mean-centering folded into the 1x1-conv weights (column-centered
    srw'') so branch conv outputs come out of PSUM already centered; the
    eps for the variance is accumulated into the stats PSUM via a K=1
    matmul, and 1/C is folded into the column-sum lhsT. x4's mean comes
    from a (-1/512)-column matmul.
  * Variance rows for all 4 LNs batched into one [4,256] sqrt +
    reciprocal; per-slot normalize is one scalar_tensor_tensor
    (x*gamma_col*rstd_bcast) + one in-place Gelu with per-partition bias.
  * Attention: scores transposed (sT[nk,nq]); softmax denominator from the
    65th ones-row of v_aug; head PAIRS share PSUM tiles so the denominators
    reciprocal is one [1,512] op per pair; proj bias via K=1 matmul and the
    proj PSUM is DMA'd straight to DRAM.
  * All input DMAs issued at batch top on the SP HWDGE queue; weights on
    the Activation HWDGE queue in first-use order; w-pools are single
    strided tensor_reduce ops alternated across DVE/Pool; everything
    double-buffered so batches pipeline.
"""

import sys

sys.path.insert(0, "/opt/trn_rl_repo")

import numpy as np

import concourse.bass as bass
import concourse.mybir as mybir
import concourse.tile as tile
from concourse.bass_utils import run_bass_kernel_spmd
from concourse.masks import make_identity

# ---------------------------------------------------------------------------
# Patch: this walrus build only accepts ONE sync-wait on a Drain instruction;
# Tile's tail drain waits on every live semaphore lane.  Split it into a chain
# of single-wait drains.
from concourse.vector_clock import ScopedClock, VectorClock
from concourse.tile_sem_assignment import N_PROCS


def _patched_drain_and_barrier(self, tick_clock, wait_clock):
    nc = self.nc
    gc = tick_clock.global_clock
    nz = [p for p in range(N_PROCS) if gc[p] > 0]
    groups = [nz[i : i + 1] for i in range(0, len(nz), 1)] or [[]]
    for g in groups[:-1]:
        masked = VectorClock([gc[p] if p in g else 0 for p in range(N_PROCS)])
        d = nc.sync.drain()
        wait_clock.add_sem_waits(d.ins, ScopedClock({None: masked}))
    drain_inst = nc.sync.drain()
    last = ScopedClock(
        {None: VectorClock([gc[p] if p in groups[-1] else 0 for p in range(N_PROCS)])}
    )
    wait_clock.add_sem_waits(drain_inst.ins, last)
    nc.all_engine_barrier()
    assert self.sems is not None
    popped = nc._tile_sem_poison_stack.pop()
    assert popped is self._sem_poison
    nc.clear_and_free_semaphores(list(self.sems.allocated().values()))
    nc.all_engine_barrier()


tile.TileContext._drain_and_barrier = _patched_drain_and_barrier


def _split_excess_waits(nc, limit=1):
    """Walrus in this build rejects >2 sync-waits on compute/DMA instructions
    (>1 on Drain).  Move excess waits onto same-engine no-ops inserted just
    before the offending instruction."""
    import bass_rust

    uid = [0]
    for f in nc.m.functions:
        for blk in f.blocks:
            newlist = []
            changed = False
            for ins in blk.instructions:
                si = ins.sync_info
                waits = list(si.on_wait) if si and si.on_wait else []
                tn = type(ins).__name__
                lim = 1 if tn in ("InstDrain", "InstNoOp", "InstTensorTensor") else limit
                if len(waits) > lim:
                    keep = waits[-lim:]
                    for w in waits[:-lim]:
                        nop = bass_rust.InstNoOp(
                            name=f"wsplit-{uid[0]}", ins=[], outs=[]
                        )
                        uid[0] += 1
                        nop.engine = ins.engine
                        nop.sync_info = mybir.SyncInfo(on_wait=[w], on_update=[])
                        newlist.append(nop)
                    ins.sync_info = mybir.SyncInfo(
                        on_wait=keep,
                        on_update=list(si.on_update) if si.on_update else [],
                    )
                    changed = True
                newlist.append(ins)
            if changed:
                blk.instructions = newlist


# ---------------------------------------------------------------------------

F32 = mybir.dt.float32
F32R = mybir.dt.float32r
BF16 = mybir.dt.bfloat16
AF = mybir.ActivationFunctionType
ALU = mybir.AluOpType
AX = mybir.AxisListType

NCORES = 8
B = 32
BPC = B // NCORES
N1 = 256
C1 = 512
NH, HD = 8, 64
SCALE = HD ** -0.5
EPS = 1e-5
C2 = (64, 128, 320)
NKV = 256
KTORD = [4, 5, 6, 7, 1, 2, 3, 0]

# xc channel-permutation (same as reference concat order -> kv-matmul kt bins):
# kt0=[x1 | x3c], kt1=x2, kt2=x3a, kt3=x3b, kt4..7=x4
_PERM = np.concatenate(
    [
        np.arange(0, 64),
        np.arange(448, 512),
        np.arange(64, 192),
        np.arange(192, 320),
        np.arange(320, 448),
        np.arange(512, 1024),
    ]
)

# ---- wsmall column layout (bf16 [128, WS]) --------------------------------
_C_AH1 = 0
_C_AH2 = 16
_C_AH3 = 48
_C_G4 = 112
_C_B4 = 116
_C_GB = 120   # 5 cols: g1(p0:64), g2, g3a, g3b, g3c(p0:64)
_C_BB = 125
_C_SEL = 130    # 4 selector mats [128, 97]: col QVAR[br] = 1/cb
_C_M4 = 518     # [128, 1] = -1/512
_C_SRBC = 519   # srb columns [128, 5]: srb1(p0:64), srb2, srb3 pt0/1/2
_R_SRB1 = 528
_R_SRB2 = 592
_R_SRB3 = 720
_R_EPS97 = 1040
_R_PJB = 1140
WS = 1664
QVAR = (0, 32, 64, 96)  # var row (psum partition) per branch y1,y2,y3,x4


def _pool_mats():
    """Ah matrices: [128, G*16] mapping partition (h,g) -> col (g*16+ho),
    with the full 1/r^2 divisor folded in."""
    out = []
    for i, (G, r) in enumerate(zip((1, 2, 4), (8, 4, 2))):
        H = 128 // G
        m = np.zeros((128, G * 16), dtype=np.float32)
        for h in range(H):
            for g in range(G):
                m[h * G + g, g * 16 + h // r] = 1.0 / (r * r)
        out.append(m)
    return out


def build_module(debug=False, reps=1):
    nc = bass.Bass(trn_type="TRN2")

    x_d = nc.dram_tensor("x", [BPC, N1, C1], F32, kind="ExternalInput")
    y1_d = nc.dram_tensor("y1", [BPC, 128 * 128, 64], F32, kind="ExternalInput")
    y2_d = nc.dram_tensor("y2", [BPC, 64 * 64, 128], F32, kind="ExternalInput")
    y3_d = nc.dram_tensor("y3", [BPC, 32 * 32, 320], F32, kind="ExternalInput")
    wsm_d = nc.dram_tensor("wsmall", [128, WS], BF16, kind="ExternalInput")
    srw_d = nc.dram_tensor("srwall", [128, 1152], BF16, kind="ExternalInput")
    wq_d = nc.dram_tensor("wqp", [128, 4, 512], BF16, kind="ExternalInput")
    wk1_d = nc.dram_tensor("wk1p", [128, 4, 1024], BF16, kind="ExternalInput")
    wk2_d = nc.dram_tensor("wk2p", [128, 4, 1024], BF16, kind="ExternalInput")
    wp_d = nc.dram_tensor("wpp", [128, 4, 512], BF16, kind="ExternalInput")
    out_d = nc.dram_tensor("out", [BPC, N1, C1], F32, kind="ExternalOutput")

    dbg_d = {}
    if debug:
        for nm, shp in [
            ("d_xT", [128, 4, NKV]), ("d_cT", [128, 4, NKV]),
            ("d_poolt1", [64, NKV]), ("d_poolt2", [128, NKV]),
            ("d_poolt3", [128, 3, NKV]), ("d_xcT", [128, 8, NKV]),
            ("d_qT", [128, 4, NKV]), ("d_kT", [128, 4, NKV]),
            ("d_vaug", [128, 2, NH, HD + 1]), ("d_outT", [128, 4, NKV]),
            ("d_sd", [4, NKV]), ("d_vstat", [8, NKV]),
        ]:
            dbg_d[nm] = nc.dram_tensor(nm, shp, F32, kind="ExternalOutput")

    with tile.TileContext(nc) as tc:
        with (
            tc.tile_pool(name="wts", bufs=1) as wts,
            tc.tile_pool(name="bands", bufs=1) as bandp,
            tc.tile_pool(name="sb", bufs=1) as sb,
            tc.tile_pool(name="rows", bufs=1) as rowsp,
            tc.tile_pool(name="pp", bufs=1, space="PSUM") as pp,
        ):
            # ---- one-time constants -----------------------------------
            ident = wts.tile([128, 128], F32)
            make_identity(nc, ident)
            ones = wts.tile([1, 1024], BF16)
            nc.vector.memset(ones, 1.0)

            # ---- weight DMAs (Act HWDGE queue), first-use order -------
            wsm = wts.tile([128, WS], BF16)
            nc.scalar.dma_start(out=wsm, in_=wsm_d.ap())
            srw_s = wts.tile([128, 1152], BF16)
            nc.scalar.dma_start(out=srw_s, in_=srw_d.ap())
            wq_s = wts.tile([128, 4, 512], BF16)
            nc.scalar.dma_start(out=wq_s, in_=wq_d.ap())
            wk1_s = wts.tile([128, 4, 1024], BF16)
            nc.scalar.dma_start(out=wk1_s, in_=wk1_d.ap())
            wk2_s = wts.tile([128, 4, 1024], BF16)
            nc.scalar.dma_start(out=wk2_s, in_=wk2_d.ap())
            wp_s = wts.tile([128, 4, 512], BF16)
            nc.scalar.dma_start(out=wp_s, in_=wp_d.ap())

            ah1 = wsm[:, _C_AH1:_C_AH1 + 16]
            ah2 = wsm[:, _C_AH2:_C_AH2 + 32]
            ah3 = wsm[:, _C_AH3:_C_AH3 + 64]
            g4c = wsm[:, _C_G4:_C_G4 + 4]
            b4c = wsm[:, _C_B4:_C_B4 + 4]
            gBc = wsm[:, _C_GB:_C_GB + 5]
            bBc = wsm[:, _C_BB:_C_BB + 5]
            sel = [wsm[:, _C_SEL + i * 97:_C_SEL + (i + 1) * 97] for i in range(4)]
            m4col = wsm[:, _C_M4:_C_M4 + 1]
            srbc = wsm[:, _C_SRBC:_C_SRBC + 5]
            srb1 = wsm[0:1, _R_SRB1:_R_SRB1 + 64]
            srb2 = wsm[0:1, _R_SRB2:_R_SRB2 + 128]
            srb3 = wsm[0:1, _R_SRB3:_R_SRB3 + 320]
            eps97 = wsm[0:1, _R_EPS97:_R_EPS97 + 97]
            pjb = wsm[0:1, _R_PJB:_R_PJB + 512]
            srw1 = srw_s[0:64, 0:64]
            srw2 = srw_s[:, 64:192]
            srw3v = srw_s[:, 192:1152].rearrange("p (t o) -> p t o", t=3)

            def wkv(kt):
                return wk1_s[:, kt - 4] if kt >= 4 else wk2_s[:, kt]

            xr = x_d.ap().rearrange("b (nt p) c -> b p nt c", p=128)
            outr = out_d.ap().rearrange("b (nt p) c -> b nt p c", p=128)
            y1r = y1_d.ap().rearrange("b (h w) c -> b h (w c)", h=128)
            y2r = y2_d.ap().rearrange("b (h wb wi) c -> b (h wb) (wi c)", wb=2, wi=32)
            y3r = y3_d.ap().rearrange("b (h wb wi) c -> b (h wb) (wi c)", wb=4, wi=8)

            MM = nc.tensor.matmul

            pend_out = []  # deferred (osb, bi) output DMAs

            def flush_out():
                while pend_out:
                    osb_p, bi_p = pend_out.pop(0)
                    nc.sync.dma_start(
                        out=outr[bi_p].rearrange("a p c -> p a c"), in_=osb_p)

            def emit_dma(bi):
                # ==== input DMA issue (SP queue) ========================
                x_sb = sb.tile([128, 2, 512], F32, tag="x_sb", bufs=2)
                nc.sync.dma_start(out=x_sb, in_=xr[bi])
                b1 = []
                for qt in range(4):
                    t = bandp.tile([128, 2048], F32, tag="bandy1", bufs=3,
                                   name=f"b1_{qt}")
                    nc.sync.dma_start(
                        out=t, in_=y1r[bi, :, qt * 2048:(qt + 1) * 2048])
                    b1.append(t)
                b2 = []
                for hf in range(2):
                    t = bandp.tile([128, 2048], F32, tag="bandy2", bufs=2,
                                   name=f"b2_{hf}")
                    nc.sync.dma_start(
                        out=t, in_=y2r[bi, :, hf * 2048:(hf + 1) * 2048])
                    b2.append(t)
                b3 = bandp.tile([128, 2560], F32, tag="bandy3", bufs=2)
                nc.sync.dma_start(out=b3, in_=y3r[bi])
                flush_out()
                return dict(bi=bi, x_sb=x_sb, b1=b1, b2=b2, b3=b3)

            def emit_fcomp(st0):
                bi = st0["bi"]
                x_sb, b1, b2, b3 = st0["x_sb"], st0["b1"], st0["b2"], st0["b3"]

                # ==== x transpose -> xT =================================
                xT = sb.tile([128, 4, NKV], BF16, tag="xT", bufs=3)
                for c2 in range(2):
                    tp = pp.tile([128, 2, 2, 128], F32, tag="ppB", bufs=2,
                                 name="xtp")
                    for cl in range(2):
                        for nt in range(2):
                            nc.tensor.transpose(
                                tp[:, cl, nt],
                                x_sb[:, nt, (c2 * 2 + cl) * 128:(c2 * 2 + cl + 1) * 128],
                                ident)
                    nc.vector.tensor_copy(
                        out=xT[:, c2 * 2:(c2 + 1) * 2].rearrange("p a b -> p (a b)"),
                        in_=tp.rearrange("p a b c -> p (a b c)"))

                # stats bank: [QVAR[br], 0] = var rows (quadrant-aligned so
                # they can be read back); [0:1, 1] = -mean(x4) row
                smf = pp.tile([128, 2, NKV], F32, tag="stats", bufs=2)
                vstat = smf[0:97, 0]
                MM(vstat, eps97, ones[:, 0:NKV], start=True, stop=True,
                   skip_group_check=True)

                # ==== q =================================================
                qT = sb.tile([128, 4, NKV], BF16, tag="qT", bufs=3)
                for mh in range(2):
                    qp = pp.tile([128, 2, NKV], F32, tag="ppB", bufs=2, name="qp")
                    for ml in range(2):
                        for kt in range(4):
                            MM(qp[:, ml],
                               wq_s[:, kt, (mh * 2 + ml) * 128:(mh * 2 + ml + 1) * 128],
                               xT[:, kt], start=(kt == 0), stop=(kt == 3))
                    nc.scalar.copy(
                        out=qT[:, mh * 2:(mh + 1) * 2].rearrange("p a b -> p (a b)"),
                        in_=qp.rearrange("p a b -> p (a b)"))

                # ==== x4 stats + centering ==============================
                for kt in range(4):
                    MM(smf[0:1, 1], m4col, xT[:, kt], start=(kt == 0),
                       stop=(kt == 3), skip_group_check=True)
                negm4 = rowsp.tile([1, NKV], BF16, tag="negm4", bufs=2)
                nc.vector.tensor_copy(out=negm4, in_=smf[0:1, 1])
                mf4 = pp.tile([128, NKV], F32, tag="ppV", bufs=2, name="mf4")
                MM(mf4, ones[:, 0:128], negm4, start=True, stop=True)
                cT = sb.tile([128, 4, NKV], BF16, tag="cT", bufs=3)
                mf4b = bass.AP(tensor=mf4.tensor, offset=mf4.offset,
                               ap=[mf4.ap[0], [0, 4], mf4.ap[1]])
                nc.vector.tensor_add(cT, xT, mf4b)
                sq4 = sb.tile([128, 4, NKV], BF16, tag="sq4", bufs=2)
                nc.vector.tensor_mul(sq4, cT, cT)
                for kt in range(4):
                    MM(vstat, sel[3], sq4[:, kt], start=False,
                       stop=(kt == 3), skip_group_check=True)

                # ==== y1: w-pool reduce + h-pool matmul + conv ==========
                t1y1 = sb.tile([128, 16, 64], BF16, tag="t1y1", bufs=2)
                pp1t = pp.tile([64, 16, 16], F32, tag="ppPC", bufs=1, name="pp1t")
                for qt in range(4):
                    v = b1[qt].rearrange("p (wo dw c) -> p wo dw c", wo=4, dw=8)
                    t05 = sb.tile([128, 4, 4, 64], BF16, tag="t05y1", bufs=2,
                                  name="t05")
                    nc.gpsimd.tensor_add(t05, v[:, :, 0:4], v[:, :, 4:8])
                    nc.vector.tensor_add(t05[:, :, 0:2], t05[:, :, 0:2],
                                         t05[:, :, 2:4])
                    nc.vector.tensor_add(t1y1[:, qt * 4:(qt + 1) * 4],
                                         t05[:, :, 0], t05[:, :, 1])
                    for wl in range(4):
                        wo = qt * 4 + wl
                        MM(pp1t[:, wo], t1y1[:, wo], ah1, start=True, stop=True)
                poolt1 = sb.tile([64, NKV], BF16, tag="poolt1", bufs=2)
                nc.scalar.copy(
                    out=poolt1, in_=pp1t.rearrange("p a b -> p (a b)"))
                prep1 = pp.tile([64, NKV], F32, tag="ppPC", bufs=1, name="prep1")
                MM(prep1, srw1, poolt1, start=True, stop=True)
                bb1 = sb.tile([64, NKV], BF16, tag="bb1", bufs=3)
                nc.scalar.activation(out=bb1, in_=prep1, func=AF.Identity,
                                     bias=srbc[0:64, 0:1])
                sq = sb.tile([128, NKV], BF16, tag="sq", bufs=4, name="sqb1")
                nc.vector.tensor_mul(sq[0:64], bb1, bb1)
                MM(vstat, sel[0][0:64], sq[0:64], start=False,
                   stop=True, skip_group_check=True)
                del sq

                # ==== y2 ================================================
                t1y2 = sb.tile([128, 8, 128], BF16, tag="t1y2", bufs=2)
                pp2t = pp.tile([128, 2, 8, 16], F32, tag="ppPC", bufs=1,
                               name="pp2t")
                for hf in range(2):
                    v = b2[hf].rearrange("p (wo dw c) -> p wo dw c", wo=4, dw=4)
                    t05 = sb.tile([128, 4, 2, 128], BF16, tag="t05y2", bufs=2,
                                  name="t05b")
                    nc.gpsimd.tensor_add(t05, v[:, :, 0:2], v[:, :, 2:4])
                    nc.vector.tensor_add(t1y2[:, hf * 4:(hf + 1) * 4],
                                         t05[:, :, 0], t05[:, :, 1])
                    for wl in range(4):
                        wo = hf * 4 + wl
                        MM(pp2t[:, :, wo], t1y2[:, wo], ah2, start=True,
                           stop=True)
                poolt2 = sb.tile([128, NKV], BF16, tag="poolt2", bufs=2)
                nc.scalar.copy(
                    out=poolt2, in_=pp2t.rearrange("p a b c -> p (a b c)"))
                prep2 = pp.tile([128, NKV], F32, tag="ppPC", bufs=1, name="prep2")
                MM(prep2, srw2, poolt2, start=True, stop=True)
                bb2 = sb.tile([128, NKV], BF16, tag="bb2", bufs=3)
                nc.scalar.activation(out=bb2, in_=prep2, func=AF.Identity,
                                     bias=srbc[:, 1:2])
                sq = sb.tile([128, NKV], BF16, tag="sq", bufs=4, name="sqb2")
                nc.vector.tensor_mul(sq, bb2, bb2)
                MM(vstat, sel[1], sq, start=False, stop=True,
                   skip_group_check=True)

                # ==== y3 ================================================
                t1y3 = sb.tile([128, 4, 320], BF16, tag="t1y3", bufs=2)
                v3 = b3.rearrange("p (wo dw c) -> p wo dw c", wo=4, dw=2)
                nc.vector.tensor_add(t1y3, v3[:, :, 0], v3[:, :, 1])
                poolt3 = sb.tile([128, 3, NKV], BF16, tag="poolt3", bufs=2)
                for cs in range(3):
                    cln = 64 if cs == 2 else 128
                    pp3 = pp.tile([128, 4, 4, 16], F32, tag="ppPC", bufs=1,
                                  name="pp3")
                    for wo in range(4):
                        MM(pp3[:cln, :, wo], t1y3[:, wo, cs * 128:cs * 128 + cln],
                           ah3, start=True, stop=True)
                    nc.scalar.copy(
                        out=poolt3[:cln, cs],
                        in_=pp3[:cln].rearrange("p a b c -> p (a b c)"))
                bb3 = sb.tile([128, 3, NKV], BF16, tag="bb3", bufs=3)
                for pt in range(3):
                    clp = 64 if pt == 2 else 128
                    prep3 = pp.tile([128, NKV], F32, tag="ppPC", bufs=1,
                                    name="prep3")
                    for kt in range(3):
                        kln = 64 if kt == 2 else 128
                        MM(prep3[:clp],
                           srw3v[:kln, kt, pt * 128:pt * 128 + clp],
                           poolt3[:kln, kt], start=(kt == 0), stop=(kt == 2))
                    nc.scalar.activation(out=bb3[:clp, pt], in_=prep3[:clp],
                                         func=AF.Identity,
                                         bias=srbc[0:clp, 2 + pt:3 + pt])
                    sq = sb.tile([128, NKV], BF16, tag="sq", bufs=4, name="sqb3")
                    nc.vector.tensor_mul(sq[0:clp], bb3[:clp, pt], bb3[:clp, pt])
                    MM(vstat, sel[2][0:clp], sq[0:clp], start=False,
                       stop=(pt == 2), skip_group_check=True)


                # LN tail: rstd = exp(-0.5 * ln(var)); ln/exp share an act
                # table with the attention exps, saving a table load per batch
                sd = rowsp.tile([1, 4, NKV], F32, tag="sd", bufs=1)
                for br in range(4):
                    nc.scalar.activation(out=sd[:, br],
                                         in_=smf[QVAR[br]:QVAR[br] + 1, 0],
                                         func=AF.Ln)
                rstd_row = rowsp.tile([1, 4, NKV], BF16, tag="rstd_row", bufs=3)
                for br in range(4):
                    nc.scalar.activation(out=rstd_row[:, br], in_=sd[:, br],
                                         func=AF.Exp, scale=-0.5)

                return dict(bi=bi, xT=xT, qT=qT, cT=cT, bb1=bb1, bb2=bb2,
                            bb3=bb3, rstd_row=rstd_row, poolt1=poolt1,
                            poolt2=poolt2, poolt3=poolt3)

            def emit_mid(st):
                bi = st["bi"]
                xT, qT, cT = st["xT"], st["qT"], st["cT"]
                bb1, bb2, bb3 = st["bb1"], st["bb2"], st["bb3"]
                rstd_row = st["rstd_row"]
                poolt1, poolt2, poolt3 = st["poolt1"], st["poolt2"], st["poolt3"]

                # rstd broadcasts (K=1 matmuls): sfB = [Sf4 | Sf2],
                # sfC = [Sf3 | Sf1], written/consumed in kv order
                sfB = pp.tile([128, 2, NKV], F32, tag="ppS", bufs=1, name="sfB")
                MM(sfB[:, 0], ones[:, 0:128], rstd_row[:, 3], start=True,
                   stop=True, skip_group_check=True)
                MM(sfB[:, 1], ones[:, 0:128], rstd_row[:, 1], start=True,
                   stop=True, skip_group_check=True)
                sfC = pp.tile([128, 2, NKV], F32, tag="ppS", bufs=1, name="sfC")
                MM(sfC[:, 0], ones[:, 0:128], rstd_row[:, 2], start=True,
                   stop=True, skip_group_check=True)
                MM(sfC[0:64, 1], ones[:, 0:64], rstd_row[:, 0], start=True,
                   stop=True, skip_group_check=True)

                # ==== normalize + gelu into xcT (kv order) ==============
                xcT = sb.tile([128, 8, NKV], BF16, tag="xcT", bufs=2)
                STT = nc.vector.scalar_tensor_tensor
                for kt in range(4):
                    tmp4 = sb.tile([128, NKV], BF16, tag="tmp4", bufs=2)
                    STT(out=tmp4, in0=cT[:, kt], scalar=g4c[:, kt:kt + 1],
                        in1=sfB[:, 0], op0=ALU.mult, op1=ALU.mult)
                    nc.scalar.activation(
                        out=xcT[:, 4 + kt].rearrange("c (wo ho) -> c ho wo", wo=16),
                        in_=tmp4.rearrange("c (ho wo) -> c ho wo", ho=16),
                        func=AF.Gelu, bias=b4c[:, kt:kt + 1])
                # y2 -> kt1
                STT(out=xcT[:, 1], in0=bb2, scalar=gBc[:, 1:2], in1=sfB[:, 1],
                    op0=ALU.mult, op1=ALU.mult)
                nc.scalar.activation(out=xcT[:, 1], in_=xcT[:, 1], func=AF.Gelu,
                                     bias=bBc[:, 1:2])
                # y3 -> kt2, kt3, kt0[64:128]
                for pt, (dst, gc_, bc_) in enumerate([
                    (xcT[:, 2], gBc[:, 2:3], bBc[:, 2:3]),
                    (xcT[:, 3], gBc[:, 3:4], bBc[:, 3:4]),
                    (xcT[64:128, 0], gBc[0:64, 4:5], bBc[0:64, 4:5]),
                ]):
                    clp = 64 if pt == 2 else 128
                    STT(out=dst, in0=bb3[:clp, pt], scalar=gc_,
                        in1=sfC[:clp, 0], op0=ALU.mult, op1=ALU.mult)
                    nc.scalar.activation(out=dst, in_=dst, func=AF.Gelu, bias=bc_)
                # y1 -> kt0[0:64]
                STT(out=xcT[0:64, 0], in0=bb1, scalar=gBc[0:64, 0:1],
                    in1=sfC[0:64, 1], op0=ALU.mult, op1=ALU.mult)
                nc.scalar.activation(out=xcT[0:64, 0], in_=xcT[0:64, 0],
                                     func=AF.Gelu, bias=bBc[0:64, 0:1])

                # ==== kv ================================================
                kT = sb.tile([128, 4, NKV], BF16, tag="kT", bufs=2)
                for mh in range(2):
                    kp = pp.tile([128, 2, NKV], F32, tag="ppB", bufs=2, name="kp")
                    for ml in range(2):
                        for i, kt in enumerate(KTORD):
                            MM(kp[:, ml],
                               wkv(kt)[:, (mh * 2 + ml) * 128:(mh * 2 + ml + 1) * 128],
                               xcT[:, kt], start=(i == 0), stop=(i == 7))
                    nc.scalar.copy(
                        out=kT[:, mh * 2:(mh + 1) * 2].rearrange("p a b -> p (a b)"),
                        in_=kp.rearrange("p a b -> p (a b)"))
                v_aug = sb.tile([128, 2, NH, HD + 1], BF16, tag="v_aug", bufs=2)
                nc.gpsimd.memset(v_aug[:, :, :, HD:HD + 1], 1.0)
                for mt in range(2):
                    vp = pp.tile([128, C1], F32, tag="ppB", bufs=2, name="vp")
                    for i, kt in enumerate(KTORD):
                        MM(vp, xcT[:, kt, mt * 128:(mt + 1) * 128],
                           wkv(kt)[:, 512:1024], start=(i == 0), stop=(i == 7))
                    nc.vector.tensor_copy(
                        out=v_aug[:, mt, :, 0:HD],
                        in_=vp.rearrange("p (h d) -> p h d", h=NH))


                st.update(dict(kT=kT, v_aug=v_aug, qT=qT, xcT=xcT))
                return st

            def emit_att(st):
                bi = st["bi"]
                kT, v_aug, qT = st["kT"], st["v_aug"], st["qT"]
                xT, cT, xcT = st["xT"], st["cT"], st["xcT"]
                poolt1, poolt2, poolt3 = st["poolt1"], st["poolt2"], st["poolt3"]
                # ==== attention (head pairs, software-interleaved) ======
                outT = sb.tile([128, 4, NKV], BF16, tag="outT", bufs=2)

                def attn_head(hp):
                    pvp = pp.tile([65, 2, NKV], F32, tag="ppV", bufs=2,
                                  name="pvp")
                    for i in range(2):
                        h = 2 * hp + i
                        pb = i * 64
                        spt = pp.tile([128, 2, NKV], F32, tag="ppB", bufs=2,
                                      name="spt")
                        for nt in range(2):
                            MM(spt[:, nt],
                               kT[pb:pb + 64, hp, nt * 128:(nt + 1) * 128],
                               qT[pb:pb + 64, hp], start=True, stop=True)
                        ste = sb.tile([128, 2, NKV], BF16, tag="ste", bufs=3)
                        nc.scalar.activation(out=ste, in_=spt, func=AF.Exp,
                                             scale=SCALE)
                        for nt in range(2):
                            MM(pvp[:, i], v_aug[:, nt, h], ste[:, nt],
                               start=(nt == 0), stop=(nt == 1))
                    # free the PSUM pair tile early via one Act copy
                    pvs = sb.tile([65, 2, NKV], BF16, tag="pvs", bufs=3)
                    nc.scalar.copy(out=pvs, in_=pvp)
                    return pvs

                def attn_tail(hp, pvs):
                    rec = rowsp.tile([1, 2 * NKV], BF16, tag="rec", bufs=3)
                    with nc.allow_low_precision(reason="tf32 softmax denom"):
                        nc.vector.reciprocal(
                            rec, pvs[64:65].rearrange("p a b -> p (a b)"))
                    bcp = pp.tile([64, 2 * NKV], F32, tag="ppB", bufs=2,
                                  name="bcp")
                    MM(bcp, ones[:, 0:64], rec, start=True, stop=True)
                    bcv = bcp.rearrange("p (a b) -> p a b", a=2)
                    for i in range(2):
                        nc.vector.tensor_mul(
                            outT[i * 64:(i + 1) * 64, hp], pvs[0:64, i],
                            bcv[:, i])

                pend = []
                for hp in range(4):
                    pend.append((hp, attn_head(hp)))
                    if len(pend) >= 4:
                        attn_tail(*pend.pop(0))
                while pend:
                    attn_tail(*pend.pop(0))

                # ==== proj + bias + store ===============================
                osb = sb.tile([128, 2, C1], F32, tag="osb", bufs=2)
                for nt in range(2):
                    fp = pp.tile([128, C1], F32, tag="ppB", bufs=2, name="fp")
                    for kt in range(4):
                        MM(fp, outT[:, kt, nt * 128:(nt + 1) * 128], wp_s[:, kt],
                           start=(kt == 0), stop=False)
                    MM(fp, ones[:, 0:128], pjb, start=False, stop=True)
                    nc.scalar.copy(out=osb[:, nt], in_=fp)
                pend_out.append((osb, bi))

                if debug and bi == 0:
                    flush_out()
                    for nm, tl in [
                        ("d_xT", xT), ("d_cT", cT), ("d_poolt1", poolt1),
                        ("d_poolt2", poolt2), ("d_poolt3", poolt3),
                        ("d_xcT", xcT), ("d_qT", qT), ("d_kT", kT),
                        ("d_vaug", v_aug), ("d_outT", outT),
                    ]:
                        tf = sb.tile(list(tl.shape), F32, tag=f"dbg{nm}",
                                     name=f"dbg{nm}")
                        nc.vector.tensor_copy(out=tf, in_=tl)
                        nc.sync.dma_start(out=dbg_d[nm].ap(), in_=tf)
                    nc.sync.dma_start(out=dbg_d["d_sd"].ap(), in_=sd)


            sts = []
            for rep in range(reps):
                for bi in range(BPC):
                    sts.append(emit_fcomp(emit_dma(bi)))
                    if len(sts) >= 2:
                        emit_att(emit_mid(sts.pop(0)))
            while sts:
                emit_att(emit_mid(sts.pop(0)))
            flush_out()

    _split_excess_waits(nc)
    return nc


def _pack_weights(inputs):
    """Host-side bf16 weight packing."""
    Wq = np.asarray(inputs["Wq"], dtype=np.float32)
    Wkv = np.asarray(inputs["Wkv"], dtype=np.float32)
    proj_w = np.asarray(inputs["proj_w"], dtype=np.float32)
    proj_b = np.asarray(inputs["proj_b"], dtype=np.float32)

    def pack_t(w, nkt, ncol):
        # w: [c_in, c_out] -> [128, nkt, ncol]
        return np.ascontiguousarray(
            w.reshape(nkt, 128, ncol).transpose(1, 0, 2))

    wq_t = Wq.T  # [512 in, 512 out]
    wkv_t = Wkv.T[_PERM, :]  # [1024 in(perm), 1024 out]
    proj_t = proj_w.T

    wqp = pack_t(wq_t, 4, 512)
    wk2p = pack_t(wkv_t[0:512], 4, 1024)
    wk1p = pack_t(wkv_t[512:1024], 4, 1024)
    wpp = pack_t(proj_t, 4, 512)

    ah = _pool_mats()
    wsm = np.zeros((128, WS), dtype=np.float32)
    wsm[:, _C_AH1:_C_AH1 + 16] = ah[0]
    wsm[:, _C_AH2:_C_AH2 + 32] = ah[1]
    wsm[:, _C_AH3:_C_AH3 + 64] = ah[2]
    g4 = np.asarray(inputs["ln4_g"], dtype=np.float32)
    b4 = np.asarray(inputs["ln4_b"], dtype=np.float32)
    wsm[:, _C_G4:_C_G4 + 4] = g4.reshape(4, 128).T
    wsm[:, _C_B4:_C_B4 + 4] = b4.reshape(4, 128).T
    for i in range(3):
        g = np.asarray(inputs[f"ln{i+1}_g"], dtype=np.float32)
        b = np.asarray(inputs[f"ln{i+1}_b"], dtype=np.float32)
        if i == 0:
            wsm[0:64, _C_GB] = g
            wsm[0:64, _C_BB] = b
        elif i == 1:
            wsm[:, _C_GB + 1] = g
            wsm[:, _C_BB + 1] = b
        else:
            wsm[:, _C_GB + 2] = g[0:128]
            wsm[:, _C_GB + 3] = g[128:256]
            wsm[0:64, _C_GB + 4] = g[256:320]
            wsm[:, _C_BB + 2] = b[0:128]
            wsm[:, _C_BB + 3] = b[128:256]
            wsm[0:64, _C_BB + 4] = b[256:320]
    for i, cb in enumerate((64, 128, 320, 512)):
        wsm[:, _C_SEL + i * 97 + QVAR[i]] = 1.0 / cb
    wsm[:, _C_M4] = -1.0 / 512
    for q in QVAR:
        wsm[0, _R_EPS97 + q] = EPS
    wsm[0, _R_PJB:_R_PJB + 512] = proj_b

    srwall = np.zeros((128, 1152), dtype=np.float32)
    srb_rows = {}
    for i, (c, off, roff) in enumerate(
            [(64, 0, _R_SRB1), (128, 64, _R_SRB2), (320, 192, _R_SRB3)]):
        w = np.asarray(inputs[f"sr{i+1}_w"], dtype=np.float32)  # [c_out, c_in]
        bsr = np.asarray(inputs[f"sr{i+1}_b"], dtype=np.float32)
        wt = w.T  # [c_in, c_out]
        wt = wt - wt.mean(axis=1, keepdims=True)  # fold LN mean-centering
        bsr = bsr - bsr.mean()
        if c == 64:
            srwall[0:64, 0:64] = wt
        elif c == 128:
            srwall[:, 64:192] = wt
        else:
            # [320, 320] -> [128, 3, 320] kt-blocks (kt2 only 64 rows)
            blk = np.zeros((128, 3, 320), dtype=np.float32)
            blk[:, 0] = wt[0:128]
            blk[:, 1] = wt[128:256]
            blk[0:64, 2] = wt[256:320]
            srwall[:, 192:1152] = blk.reshape(128, 960)
        wsm[0, roff:roff + c] = bsr
        if c == 64:
            wsm[0:64, _C_SRBC] = bsr
        elif c == 128:
            wsm[:, _C_SRBC + 1] = bsr
        else:
            wsm[:, _C_SRBC + 2] = bsr[0:128]
            wsm[:, _C_SRBC + 3] = bsr[128:256]
            wsm[0:64, _C_SRBC + 4] = bsr[256:320]
        srb_rows[i] = bsr

    import ml_dtypes

    def cast(a):
        return np.ascontiguousarray(a.astype(ml_dtypes.bfloat16))

    return {
        "wsmall": cast(wsm),
        "srwall": cast(srwall),
        "wqp": cast(wqp),
        "wk1p": cast(wk1p),
        "wk2p": cast(wk2p),
        "wpp": cast(wpp),
    }


def kernel(**inputs):
    x = np.ascontiguousarray(inputs["x"], dtype=np.float32)
    y1 = np.ascontiguousarray(inputs["y1"], dtype=np.float32)
    y2 = np.ascontiguousarray(inputs["y2"], dtype=np.float32)
    y3 = np.ascontiguousarray(inputs["y3"], dtype=np.float32)

    common = _pack_weights(inputs)

    nc = build_module()
    in_maps = []
    for c in range(NCORES):
        sl = slice(c * BPC, (c + 1) * BPC)
        m = dict(common)
        m["x"] = x[sl]
        m["y1"] = y1[sl]
        m["y2"] = y2[sl]
        m["y3"] = y3[sl]
        in_maps.append(m)

    res = run_bass_kernel_spmd(nc, in_maps, core_ids=list(range(NCORES)))
    return np.concatenate([r["out"] for r in res.results], axis=0)


if __name__ == "__main__":
    pass


# revision 45
# speedup vs baseline: 43503.5914x; 1.0071x over previous
"""Trainium2 Bass kernel for nn_MultiCrossAttention (PVT-style multi-scale
spatial-reduction cross attention).

Sharding: data-parallel over batch (B=32 -> 4 per core x 8 cores), weights
replicated.

v2 design (vs baseline):
  * bf16 weights packed host-side (halves weight DMA + SBUF, guarantees
    1 cyc/row matmuls); activations copied out of PSUM as bf16.
  * LN mean-centering folded into the 1x1-conv weights (column-centered
    srw'') so branch conv outputs come out of PSUM already centered; the
    eps for the variance is accumulated into the stats PSUM via a K=1
    matmul, and 1/C is folded into the column-sum lhsT. x4's mean comes
    from a (-1/512)-column matmul.
  * Variance rows for all 4 LNs batched into one [4,256] sqrt +
    reciprocal; per-slot normalize is one scalar_tensor_tensor
    (x*gamma_col*rstd_bcast) + one in-place Gelu with per-partition bias.
  * Attention: scores transposed (sT[nk,nq]); softmax denominator from the
    65th ones-row of v_aug; head PAIRS share PSUM tiles so the denominators
    reciprocal is one [1,512] op per pair; proj bias via K=1 matmul and the
    proj PSUM is DMA'd straight to DRAM.
  * All input DMAs issued at batch top on the SP HWDGE queue; weights on
    the Activation HWDGE queue in first-use order; w-pools are single
    strided tensor_reduce ops alternated across DVE/Pool; everything
    double-buffered so batches pipeline.
"""

import sys

sys.path.insert(0, "/opt/trn_rl_repo")

import numpy as np

import concourse.bass as bass
import concourse.mybir as mybir
import concourse.tile as tile
from concourse.bass_utils import run_bass_kernel_spmd
from concourse.masks import make_identity

# ---------------------------------------------------------------------------
# Patch: this walrus build only accepts ONE sync-wait on a Drain instruction;
# Tile's tail drain waits on every live semaphore lane.  Split it into a chain
# of single-wait drains.
from concourse.vector_clock import ScopedClock, VectorClock
from concourse.tile_sem_assignment import N_PROCS


def _patched_drain_and_barrier(self, tick_clock, wait_clock):
    nc = self.nc
    gc = tick_clock.global_clock
    nz = [p for p in range(N_PROCS) if gc[p] > 0]
    groups = [nz[i : i + 1] for i in range(0, len(nz), 1)] or [[]]
    for g in groups[:-1]:
        masked = VectorClock([gc[p] if p in g else 0 for p in range(N_PROCS)])
        d = nc.sync.drain()
        wait_clock.add_sem_waits(d.ins, ScopedClock({None: masked}))
    drain_inst = nc.sync.drain()
    last = ScopedClock(
        {None: VectorClock([gc[p] if p in groups[-1] else 0 for p in range(N_PROCS)])}
    )
    wait_clock.add_sem_waits(drain_inst.ins, last)
    nc.all_engine_barrier()
    assert self.sems is not None
    popped = nc._tile_sem_poison_stack.pop()
    assert popped is self._sem_poison
    nc.clear_and_free_semaphores(list(self.sems.allocated().values()))
    nc.all_engine_barrier()


tile.TileContext._drain_and_barrier = _patched_drain_and_barrier


def _split_excess_waits(nc, limit=1):
    """Walrus in this build rejects >2 sync-waits on compute/DMA instructions
    (>1 on Drain).  Move excess waits onto same-engine no-ops inserted just
    before the offending instruction."""
    import bass_rust

    uid = [0]
    for f in nc.m.functions:
        for blk in f.blocks:
            newlist = []
            changed = False
            for ins in blk.instructions:
                si = ins.sync_info
                waits = list(si.on_wait) if si and si.on_wait else []
                tn = type(ins).__name__
                lim = 1 if tn in ("InstDrain", "InstNoOp", "InstTensorTensor") else limit
                if len(waits) > lim:
                    keep = waits[-lim:]
                    for w in waits[:-lim]:
                        nop = bass_rust.InstNoOp(
                            name=f"wsplit-{uid[0]}", ins=[], outs=[]
                        )
                        uid[0] += 1
                        nop.engine = ins.engine
                        nop.sync_info = mybir.SyncInfo(on_wait=[w], on_update=[])
                        newlist.append(nop)
                    ins.sync_info = mybir.SyncInfo(
                        on_wait=keep,
                        on_update=list(si.on_update) if si.on_update else [],
                    )
                    changed = True
                newlist.append(ins)
            if changed:
                blk.instructions = newlist


# ---------------------------------------------------------------------------

F32 = mybir.dt.float32
F32R = mybir.dt.float32r
BF16 = mybir.dt.bfloat16
AF = mybir.ActivationFunctionType
ALU = mybir.AluOpType
AX = mybir.AxisListType

NCORES = 8
B = 32
BPC = B // NCORES
N1 = 256
C1 = 512
NH, HD = 8, 64
SCALE = HD ** -0.5
EPS = 1e-5
C2 = (64, 128, 320)
NKV = 256
KTORD = [4, 5, 6, 7, 1, 2, 3, 0]

# xc channel-permutation (same as reference concat order -> kv-matmul kt bins):
# kt0=[x1 | x3c], kt1=x2, kt2=x3a, kt3=x3b, kt4..7=x4
_PERM = np.concatenate(
    [
        np.arange(0, 64),
        np.arange(448, 512),
        np.arange(64, 192),
        np.arange(192, 320),
        np.arange(320, 448),
        np.arange(512, 1024),
    ]
)

# ---- wsmall column layout (bf16 [128, WS]) --------------------------------
_C_AH1 = 0
_C_AH2 = 16
_C_AH3 = 48
_C_G4 = 112
_C_B4 = 116
_C_GB = 120   # 5 cols: g1(p0:64), g2, g3a, g3b, g3c(p0:64)
_C_BB = 125
_C_INVCB = 130  # [128, 4] columns: 1/64, 1/128, 1/320, 1/512
_C_M4 = 518     # [128, 1] = -1/512
_C_SRBC = 519   # srb columns [128, 5]: srb1(p0:64), srb2, srb3 pt0/1/2
_R_SRB1 = 528
_R_SRB2 = 592
_R_SRB3 = 720
_R_EPS512 = 1040  # [1, 512] = EPS everywhere
_R_PJB = 1564
WS = 2088


def _pool_mats():
    """Ah matrices: [128, G*16] mapping partition (h,g) -> col (g*16+ho),
    with the full 1/r^2 divisor folded in."""
    out = []
    for i, (G, r) in enumerate(zip((1, 2, 4), (8, 4, 2))):
        H = 128 // G
        m = np.zeros((128, G * 16), dtype=np.float32)
        for h in range(H):
            for g in range(G):
                m[h * G + g, g * 16 + h // r] = 1.0 / (r * r)
        out.append(m)
    return out


def build_module(debug=False, reps=1):
    nc = bass.Bass(trn_type="TRN2")

    x_d = nc.dram_tensor("x", [BPC, N1, C1], F32, kind="ExternalInput")
    y1_d = nc.dram_tensor("y1", [BPC, 128 * 128, 64], F32, kind="ExternalInput")
    y2_d = nc.dram_tensor("y2", [BPC, 64 * 64, 128], F32, kind="ExternalInput")
    y3_d = nc.dram_tensor("y3", [BPC, 32 * 32, 320], F32, kind="ExternalInput")
    wsm_d = nc.dram_tensor("wsmall", [128, WS], BF16, kind="ExternalInput")
    srw_d = nc.dram_tensor("srwall", [128, 1152], BF16, kind="ExternalInput")
    wq_d = nc.dram_tensor("wqp", [128, 4, 512], BF16, kind="ExternalInput")
    wk1_d = nc.dram_tensor("wk1p", [128, 4, 1024], BF16, kind="ExternalInput")
    wk2_d = nc.dram_tensor("wk2p", [128, 4, 1024], BF16, kind="ExternalInput")
    wp_d = nc.dram_tensor("wpp", [128, 4, 512], BF16, kind="ExternalInput")
    out_d = nc.dram_tensor("out", [BPC, N1, C1], F32, kind="ExternalOutput")

    dbg_d = {}
    if debug:
        for nm, shp in [
            ("d_xT", [128, 4, NKV]), ("d_cT", [128, 4, NKV]),
            ("d_poolt1", [64, NKV]), ("d_poolt2", [128, NKV]),
            ("d_poolt3", [128, 3, NKV]), ("d_xcT", [128, 8, NKV]),
            ("d_qT", [128, 4, NKV]), ("d_kT", [128, 4, NKV]),
            ("d_vaug", [128, 2, NH, HD + 1]), ("d_outT", [128, 4, NKV]),
            ("d_sd", [4, NKV]), ("d_vstat", [8, NKV]),
        ]:
            dbg_d[nm] = nc.dram_tensor(nm, shp, F32, kind="ExternalOutput")

    with tile.TileContext(nc) as tc:
        with (
            tc.tile_pool(name="wts", bufs=1) as wts,
            tc.tile_pool(name="bands", bufs=1) as bandp,
            tc.tile_pool(name="sb", bufs=1) as sb,
            tc.tile_pool(name="rows", bufs=1) as rowsp,
            tc.tile_pool(name="pp", bufs=1, space="PSUM") as pp,
        ):
            # ---- one-time constants -----------------------------------
            ident = wts.tile([128, 128], F32)
            make_identity(nc, ident)
            ones = wts.tile([1, 1024], BF16)
            nc.vector.memset(ones, 1.0)

            # ---- weight DMAs (Act HWDGE queue), first-use order -------
            wsm = wts.tile([128, WS], BF16)
            nc.scalar.dma_start(out=wsm, in_=wsm_d.ap())
            srw_s = wts.tile([128, 1152], BF16)
            nc.scalar.dma_start(out=srw_s, in_=srw_d.ap())
            wq_s = wts.tile([128, 4, 512], BF16)
            nc.scalar.dma_start(out=wq_s, in_=wq_d.ap())
            wk1_s = wts.tile([128, 4, 1024], BF16)
            nc.scalar.dma_start(out=wk1_s, in_=wk1_d.ap())
            wk2_s = wts.tile([128, 4, 1024], BF16)
            nc.scalar.dma_start(out=wk2_s, in_=wk2_d.ap())
            wp_s = wts.tile([128, 4, 512], BF16)
            nc.scalar.dma_start(out=wp_s, in_=wp_d.ap())

            ah1 = wsm[:, _C_AH1:_C_AH1 + 16]
            ah2 = wsm[:, _C_AH2:_C_AH2 + 32]
            ah3 = wsm[:, _C_AH3:_C_AH3 + 64]
            g4c = wsm[:, _C_G4:_C_G4 + 4]
            b4c = wsm[:, _C_B4:_C_B4 + 4]
            gBc = wsm[:, _C_GB:_C_GB + 5]
            bBc = wsm[:, _C_BB:_C_BB + 5]
            invcb = wsm[:, _C_INVCB:_C_INVCB + 4]
            m4col = wsm[:, _C_M4:_C_M4 + 1]
            srbc = wsm[:, _C_SRBC:_C_SRBC + 5]
            srb1 = wsm[0:1, _R_SRB1:_R_SRB1 + 64]
            srb2 = wsm[0:1, _R_SRB2:_R_SRB2 + 128]
            srb3 = wsm[0:1, _R_SRB3:_R_SRB3 + 320]
            eps512 = wsm[0:1, _R_EPS512:_R_EPS512 + 512]
            pjb = wsm[0:1, _R_PJB:_R_PJB + 512]
            srw1 = srw_s[0:64, 0:64]
            srw2 = srw_s[:, 64:192]
            srw3v = srw_s[:, 192:1152].rearrange("p (t o) -> p t o", t=3)

            def wkv(kt):
                return wk1_s[:, kt - 4] if kt >= 4 else wk2_s[:, kt]

            xr = x_d.ap().rearrange("b (nt p) c -> b p nt c", p=128)
            outr = out_d.ap().rearrange("b (nt p) c -> b nt p c", p=128)
            y1r = y1_d.ap().rearrange("b (h w) c -> b h (w c)", h=128)
            y2r = y2_d.ap().rearrange("b (h wb wi) c -> b (h wb) (wi c)", wb=2, wi=32)
            y3r = y3_d.ap().rearrange("b (h wb wi) c -> b (h wb) (wi c)", wb=4, wi=8)

            MM = nc.tensor.matmul

            pend_out = []  # deferred (osb, bi) output DMAs

            def flush_out():
                while pend_out:
                    osb_p, bi_p = pend_out.pop(0)
                    nc.sync.dma_start(
                        out=outr[bi_p].rearrange("a p c -> p a c"), in_=osb_p)

            def emit_dma(bi):
                # ==== input DMA issue (SP queue) ========================
                x_sb = sb.tile([128, 2, 512], F32, tag="x_sb", bufs=2)
                nc.sync.dma_start(out=x_sb, in_=xr[bi])
                b1 = []
                for qt in range(4):
                    t = bandp.tile([128, 2048], F32, tag="bandy1", bufs=3,
                                   name=f"b1_{qt}")
                    nc.sync.dma_start(
                        out=t, in_=y1r[bi, :, qt * 2048:(qt + 1) * 2048])
                    b1.append(t)
                b2 = []
                for hf in range(2):
                    t = bandp.tile([128, 2048], F32, tag="bandy2", bufs=2,
                                   name=f"b2_{hf}")
                    nc.sync.dma_start(
                        out=t, in_=y2r[bi, :, hf * 2048:(hf + 1) * 2048])
                    b2.append(t)
                b3 = bandp.tile([128, 2560], F32, tag="bandy3", bufs=2)
                nc.sync.dma_start(out=b3, in_=y3r[bi])
                flush_out()
                return dict(bi=bi, x_sb=x_sb, b1=b1, b2=b2, b3=b3)

            def emit_fcomp(st0):
                bi = st0["bi"]
                x_sb, b1, b2, b3 = st0["x_sb"], st0["b1"], st0["b2"], st0["b3"]

                # ==== x transpose -> xT =================================
                xT = sb.tile([128, 4, NKV], BF16, tag="xT", bufs=3)
                for c2 in range(2):
                    tp = pp.tile([128, 2, 2, 128], F32, tag="ppB", bufs=2,
                                 name="xtp")
                    for cl in range(2):
                        for nt in range(2):
                            nc.tensor.transpose(
                                tp[:, cl, nt],
                                x_sb[:, nt, (c2 * 2 + cl) * 128:(c2 * 2 + cl + 1) * 128],
                                ident)
                    nc.vector.tensor_copy(
                        out=xT[:, c2 * 2:(c2 + 1) * 2].rearrange("p a b -> p (a b)"),
                        in_=tp.rearrange("p a b c -> p (a b c)"))

                # stats bank: [0, 0|1] = var(y1|y2); [32, 0|1] = var(y3|x4);
                # [64, 0] = -mean(x4).  Two-per-quadrant packing so the
                # ln/exp rstd ops cover two branches each.
                smf = pp.tile([128, 2, NKV], F32, tag="stats", bufs=2)
                MM(smf[0:1, 0:2].rearrange("p a b -> p (a b)"), ones[0:1, 0:1],
                   eps512, start=True, stop=True, skip_group_check=True)
                MM(smf[32:33, 0:2].rearrange("p a b -> p (a b)"), ones[0:1, 0:1],
                   eps512, start=True, stop=True, skip_group_check=True)

                # ==== q =================================================
                qT = sb.tile([128, 4, NKV], BF16, tag="qT", bufs=3)
                for mh in range(2):
                    qp = pp.tile([128, 2, NKV], F32, tag="ppB", bufs=2, name="qp")
                    for ml in range(2):
                        for kt in range(4):
                            MM(qp[:, ml],
                               wq_s[:, kt, (mh * 2 + ml) * 128:(mh * 2 + ml + 1) * 128],
                               xT[:, kt], start=(kt == 0), stop=(kt == 3))
                    nc.scalar.copy(
                        out=qT[:, mh * 2:(mh + 1) * 2].rearrange("p a b -> p (a b)"),
                        in_=qp.rearrange("p a b -> p (a b)"))

                # ==== x4 stats + centering ==============================
                for kt in range(4):
                    MM(smf[64:65, 0], m4col, xT[:, kt], start=(kt == 0),
                       stop=(kt == 3), skip_group_check=True)
                negm4 = rowsp.tile([1, NKV], BF16, tag="negm4", bufs=2)
                nc.vector.tensor_copy(out=negm4, in_=smf[64:65, 0])
                mf4 = pp.tile([128, NKV], F32, tag="ppV", bufs=2, name="mf4")
                MM(mf4, ones[:, 0:128], negm4, start=True, stop=True)
                cT = sb.tile([128, 4, NKV], BF16, tag="cT", bufs=3)
                mf4b = bass.AP(tensor=mf4.tensor, offset=mf4.offset,
                               ap=[mf4.ap[0], [0, 4], mf4.ap[1]])
                nc.vector.tensor_add(cT, xT, mf4b)
                sq4 = sb.tile([128, 4, NKV], BF16, tag="sq4", bufs=2)
                nc.vector.tensor_mul(sq4, cT, cT)
                for kt in range(4):
                    MM(smf[32:33, 1], invcb[:, 3:4], sq4[:, kt], start=False,
                       stop=(kt == 3), skip_group_check=True)

                # ==== y1: w-pool reduce + h-pool matmul + conv ==========
                t1y1 = sb.tile([128, 16, 64], BF16, tag="t1y1", bufs=2)
                pp1t = pp.tile([64, 16, 16], F32, tag="ppPC", bufs=1, name="pp1t")
                for qt in range(4):
                    v = b1[qt].rearrange("p (wo dw c) -> p wo dw c", wo=4, dw=8)
                    t05 = sb.tile([128, 4, 4, 64], BF16, tag="t05y1", bufs=2,
                                  name="t05")
                    nc.gpsimd.tensor_add(t05, v[:, :, 0:4], v[:, :, 4:8])
                    nc.vector.tensor_add(t05[:, :, 0:2], t05[:, :, 0:2],
                                         t05[:, :, 2:4])
                    nc.vector.tensor_add(t1y1[:, qt * 4:(qt + 1) * 4],
                                         t05[:, :, 0], t05[:, :, 1])
                    for wl in range(4):
                        wo = qt * 4 + wl
                        MM(pp1t[:, wo], t1y1[:, wo], ah1, start=True, stop=True)
                poolt1 = sb.tile([64, NKV], BF16, tag="poolt1", bufs=2)
                nc.scalar.copy(
                    out=poolt1, in_=pp1t.rearrange("p a b -> p (a b)"))
                prep1 = pp.tile([64, NKV], F32, tag="ppPC", bufs=1, name="prep1")
                MM(prep1, srw1, poolt1, start=True, stop=True)
                bb1 = sb.tile([64, NKV], BF16, tag="bb1", bufs=3)
                nc.scalar.activation(out=bb1, in_=prep1, func=AF.Identity,
                                     bias=srbc[0:64, 0:1])
                sq = sb.tile([128, NKV], BF16, tag="sq", bufs=4, name="sqb1")
                nc.vector.tensor_mul(sq[0:64], bb1, bb1)
                MM(smf[0:1, 0], invcb[0:64, 0:1], sq[0:64], start=False,
                   stop=True, skip_group_check=True)
                del sq

                # ==== y2 ================================================
                t1y2 = sb.tile([128, 8, 128], BF16, tag="t1y2", bufs=2)
                pp2t = pp.tile([128, 2, 8, 16], F32, tag="ppPC", bufs=1,
                               name="pp2t")
                for hf in range(2):
                    v = b2[hf].rearrange("p (wo dw c) -> p wo dw c", wo=4, dw=4)
                    t05 = sb.tile([128, 4, 2, 128], BF16, tag="t05y2", bufs=2,
                                  name="t05b")
                    nc.gpsimd.tensor_add(t05, v[:, :, 0:2], v[:, :, 2:4])
                    nc.vector.tensor_add(t1y2[:, hf * 4:(hf + 1) * 4],
                                         t05[:, :, 0], t05[:, :, 1])
                    for wl in range(4):
                        wo = hf * 4 + wl
                        MM(pp2t[:, :, wo], t1y2[:, wo], ah2, start=True,
                           stop=True)
                poolt2 = sb.tile([128, NKV], BF16, tag="poolt2", bufs=2)
                nc.scalar.copy(
                    out=poolt2, in_=pp2t.rearrange("p a b c -> p (a b c)"))
                prep2 = pp.tile([128, NKV], F32, tag="ppPC", bufs=1, name="prep2")
                MM(prep2, srw2, poolt2, start=True, stop=True)
                bb2 = sb.tile([128, NKV], BF16, tag="bb2", bufs=3)
                nc.scalar.activation(out=bb2, in_=prep2, func=AF.Identity,
                                     bias=srbc[:, 1:2])
                sq = sb.tile([128, NKV], BF16, tag="sq", bufs=4, name="sqb2")
                nc.vector.tensor_mul(sq, bb2, bb2)
                MM(smf[0:1, 1], invcb[:, 1:2], sq, start=False, stop=True,
                   skip_group_check=True)

                # ==== y3 ================================================
                t1y3 = sb.tile([128, 4, 320], BF16, tag="t1y3", bufs=2)
                v3 = b3.rearrange("p (wo dw c) -> p wo dw c", wo=4, dw=2)
                nc.vector.tensor_add(t1y3, v3[:, :, 0], v3[:, :, 1])
                poolt3 = sb.tile([128, 3, NKV], BF16, tag="poolt3", bufs=2)
                for cs in range(3):
                    cln = 64 if cs == 2 else 128
                    pp3 = pp.tile([128, 4, 4, 16], F32, tag="ppPC", bufs=1,
                                  name="pp3")
                    for wo in range(4):
                        MM(pp3[:cln, :, wo], t1y3[:, wo, cs * 128:cs * 128 + cln],
                           ah3, start=True, stop=True)
                    nc.scalar.copy(
                        out=poolt3[:cln, cs],
                        in_=pp3[:cln].rearrange("p a b c -> p (a b c)"))
                bb3 = sb.tile([128, 3, NKV], BF16, tag="bb3", bufs=3)
                for pt in range(3):
                    clp = 64 if pt == 2 else 128
                    prep3 = pp.tile([128, NKV], F32, tag="ppPC", bufs=1,
                                    name="prep3")
                    for kt in range(3):
                        kln = 64 if kt == 2 else 128
                        MM(prep3[:clp],
                           srw3v[:kln, kt, pt * 128:pt * 128 + clp],
                           poolt3[:kln, kt], start=(kt == 0), stop=(kt == 2))
                    nc.scalar.activation(out=bb3[:clp, pt], in_=prep3[:clp],
                                         func=AF.Identity,
                                         bias=srbc[0:clp, 2 + pt:3 + pt])
                    sq = sb.tile([128, NKV], BF16, tag="sq", bufs=4, name="sqb3")
                    nc.vector.tensor_mul(sq[0:clp], bb3[:clp, pt], bb3[:clp, pt])
                    MM(smf[32:33, 0], invcb[0:clp, 2:3], sq[0:clp],
                       start=False, stop=(pt == 2), skip_group_check=True)


                # LN tail: rstd = exp(-0.5 * ln(var)); ln/exp share an act
                # table with the attention exps.  Two branches per op.
                sd = rowsp.tile([1, 4, NKV], F32, tag="sd", bufs=1)
                nc.scalar.activation(
                    out=sd[:, 0:2].rearrange("p a b -> p (a b)"),
                    in_=smf[0:1, 0:2].rearrange("p a b -> p (a b)"), func=AF.Ln)
                nc.scalar.activation(
                    out=sd[:, 2:4].rearrange("p a b -> p (a b)"),
                    in_=smf[32:33, 0:2].rearrange("p a b -> p (a b)"),
                    func=AF.Ln)
                rstd_row = rowsp.tile([1, 4, NKV], BF16, tag="rstd_row", bufs=3)
                nc.scalar.activation(
                    out=rstd_row[:, 0:2].rearrange("p a b -> p (a b)"),
                    in_=sd[:, 0:2].rearrange("p a b -> p (a b)"), func=AF.Exp,
                    scale=-0.5)
                nc.scalar.activation(
                    out=rstd_row[:, 2:4].rearrange("p a b -> p (a b)"),
                    in_=sd[:, 2:4].rearrange("p a b -> p (a b)"), func=AF.Exp,
                    scale=-0.5)

                return dict(bi=bi, xT=xT, qT=qT, cT=cT, bb1=bb1, bb2=bb2,
                            bb3=bb3, rstd_row=rstd_row, poolt1=poolt1,
                            poolt2=poolt2, poolt3=poolt3)

            def emit_mid(st):
                bi = st["bi"]
                xT, qT, cT = st["xT"], st["qT"], st["cT"]
                bb1, bb2, bb3 = st["bb1"], st["bb2"], st["bb3"]
                rstd_row = st["rstd_row"]
                poolt1, poolt2, poolt3 = st["poolt1"], st["poolt2"], st["poolt3"]

                # rstd broadcasts (K=1 matmuls): sfB = [Sf4 | Sf2],
                # sfC = [Sf3 | Sf1], written/consumed in kv order
                sfB = pp.tile([128, 2, NKV], F32, tag="ppS", bufs=1, name="sfB")
                MM(sfB[:, 0], ones[:, 0:128], rstd_row[:, 3], start=True,
                   stop=True, skip_group_check=True)
                MM(sfB[:, 1], ones[:, 0:128], rstd_row[:, 1], start=True,
                   stop=True, skip_group_check=True)
                sfC = pp.tile([128, 2, NKV], F32, tag="ppS", bufs=1, name="sfC")
                MM(sfC[:, 0], ones[:, 0:128], rstd_row[:, 2], start=True,
                   stop=True, skip_group_check=True)
                MM(sfC[0:64, 1], ones[:, 0:64], rstd_row[:, 0], start=True,
                   stop=True, skip_group_check=True)

                # ==== normalize + gelu into xcT (kv order) ==============
                xcT = sb.tile([128, 8, NKV], BF16, tag="xcT", bufs=2)
                STT = nc.vector.scalar_tensor_tensor
                for kt in range(4):
                    tmp4 = sb.tile([128, NKV], BF16, tag="tmp4", bufs=2)
                    STT(out=tmp4, in0=cT[:, kt], scalar=g4c[:, kt:kt + 1],
                        in1=sfB[:, 0], op0=ALU.mult, op1=ALU.mult)
                    nc.scalar.activation(
                        out=xcT[:, 4 + kt].rearrange("c (wo ho) -> c ho wo", wo=16),
                        in_=tmp4.rearrange("c (ho wo) -> c ho wo", ho=16),
                        func=AF.Gelu, bias=b4c[:, kt:kt + 1])
                # y2 -> kt1
                STT(out=xcT[:, 1], in0=bb2, scalar=gBc[:, 1:2], in1=sfB[:, 1],
                    op0=ALU.mult, op1=ALU.mult)
                nc.scalar.activation(out=xcT[:, 1], in_=xcT[:, 1], func=AF.Gelu,
                                     bias=bBc[:, 1:2])
                # y3 -> kt2, kt3, kt0[64:128]
                for pt, (dst, gc_, bc_) in enumerate([
                    (xcT[:, 2], gBc[:, 2:3], bBc[:, 2:3]),
                    (xcT[:, 3], gBc[:, 3:4], bBc[:, 3:4]),
                    (xcT[64:128, 0], gBc[0:64, 4:5], bBc[0:64, 4:5]),
                ]):
                    clp = 64 if pt == 2 else 128
                    STT(out=dst, in0=bb3[:clp, pt], scalar=gc_,
                        in1=sfC[:clp, 0], op0=ALU.mult, op1=ALU.mult)
                    nc.scalar.activation(out=dst, in_=dst, func=AF.Gelu, bias=bc_)
                # y1 -> kt0[0:64]
                STT(out=xcT[0:64, 0], in0=bb1, scalar=gBc[0:64, 0:1],
                    in1=sfC[0:64, 1], op0=ALU.mult, op1=ALU.mult)
                nc.scalar.activation(out=xcT[0:64, 0], in_=xcT[0:64, 0],
                                     func=AF.Gelu, bias=bBc[0:64, 0:1])

                # ==== kv ================================================
                kT = sb.tile([128, 4, NKV], BF16, tag="kT", bufs=2)
                for mh in range(2):
                    kp = pp.tile([128, 2, NKV], F32, tag="ppB", bufs=2, name="kp")
                    for ml in range(2):
                        for i, kt in enumerate(KTORD):
                            MM(kp[:, ml],
                               wkv(kt)[:, (mh * 2 + ml) * 128:(mh * 2 + ml + 1) * 128],
                               xcT[:, kt], start=(i == 0), stop=(i == 7))
                    nc.scalar.copy(
                        out=kT[:, mh * 2:(mh + 1) * 2].rearrange("p a b -> p (a b)"),
                        in_=kp.rearrange("p a b -> p (a b)"))
                v_aug = sb.tile([128, 2, NH, HD + 1], BF16, tag="v_aug", bufs=2)
                nc.gpsimd.memset(v_aug[:, :, :, HD:HD + 1], 1.0)
                for mt in range(2):
                    vp = pp.tile([128, C1], F32, tag="ppB", bufs=2, name="vp")
                    for i, kt in enumerate(KTORD):
                        MM(vp, xcT[:, kt, mt * 128:(mt + 1) * 128],
                           wkv(kt)[:, 512:1024], start=(i == 0), stop=(i == 7))
                    nc.vector.tensor_copy(
                        out=v_aug[:, mt, :, 0:HD],
                        in_=vp.rearrange("p (h d) -> p h d", h=NH))


                st.update(dict(kT=kT, v_aug=v_aug, qT=qT, xcT=xcT))
                return st

            def emit_att(st):
                bi = st["bi"]
                kT, v_aug, qT = st["kT"], st["v_aug"], st["qT"]
                xT, cT, xcT = st["xT"], st["cT"], st["xcT"]
                poolt1, poolt2, poolt3 = st["poolt1"], st["poolt2"], st["poolt3"]
                # ==== attention (head pairs, software-interleaved) ======
                outT = sb.tile([128, 4, NKV], BF16, tag="outT", bufs=2)

                def attn_head(hp):
                    pvp = pp.tile([65, 2, NKV], F32, tag="ppV", bufs=2,
                                  name="pvp")
                    for i in range(2):
                        h = 2 * hp + i
                        pb = i * 64
                        spt = pp.tile([128, 2, NKV], F32, tag="ppB", bufs=2,
                                      name="spt")
                        for nt in range(2):
                            MM(spt[:, nt],
                               kT[pb:pb + 64, hp, nt * 128:(nt + 1) * 128],
                               qT[pb:pb + 64, hp], start=True, stop=True)
                        ste = sb.tile([128, 2, NKV], BF16, tag="ste", bufs=3)
                        nc.scalar.activation(out=ste, in_=spt, func=AF.Exp,
                                             scale=SCALE)
                        for nt in range(2):
                            MM(pvp[:, i], v_aug[:, nt, h], ste[:, nt],
                               start=(nt == 0), stop=(nt == 1))
                    # free the PSUM pair tile early via one Act copy
                    pvs = sb.tile([65, 2, NKV], BF16, tag="pvs", bufs=3)
                    nc.scalar.copy(out=pvs, in_=pvp)
                    return pvs

                def attn_tail(hp, pvs):
                    rec = rowsp.tile([1, 2 * NKV], BF16, tag="rec", bufs=3)
                    with nc.allow_low_precision(reason="tf32 softmax denom"):
                        nc.vector.reciprocal(
                            rec, pvs[64:65].rearrange("p a b -> p (a b)"))
                    bcp = pp.tile([64, 2 * NKV], F32, tag="ppB", bufs=2,
                                  name="bcp")
                    MM(bcp, ones[:, 0:64], rec, start=True, stop=True)
                    bcv = bcp.rearrange("p (a b) -> p a b", a=2)
                    for i in range(2):
                        nc.vector.tensor_mul(
                            outT[i * 64:(i + 1) * 64, hp], pvs[0:64, i],
                            bcv[:, i])

                pend = []
                for hp in range(4):
                    pend.append((hp, attn_head(hp)))
                    if len(pend) >= 4:
                        attn_tail(*pend.pop(0))
                while pend:
                    attn_tail(*pend.pop(0))

                # ==== proj + bias + store ===============================
                osb = sb.tile([128, 2, C1], F32, tag="osb", bufs=2)
                for nt in range(2):
                    fp = pp.tile([128, C1], F32, tag="ppB", bufs=2, name="fp")
                    for kt in range(4):
                        MM(fp, outT[:, kt, nt * 128:(nt + 1) * 128], wp_s[:, kt],
                           start=(kt == 0), stop=False)
                    MM(fp, ones[:, 0:128], pjb, start=False, stop=True)
                    nc.scalar.copy(out=osb[:, nt], in_=fp)
                pend_out.append((osb, bi))

                if debug and bi == 0:
                    flush_out()
                    for nm, tl in [
                        ("d_xT", xT), ("d_cT", cT), ("d_poolt1", poolt1),
                        ("d_poolt2", poolt2), ("d_poolt3", poolt3),
                        ("d_xcT", xcT), ("d_qT", qT), ("d_kT", kT),
                        ("d_vaug", v_aug), ("d_outT", outT),
                    ]:
                        tf = sb.tile(list(tl.shape), F32, tag=f"dbg{nm}",
                                     name=f"dbg{nm}")
                        nc.vector.tensor_copy(out=tf, in_=tl)
                        nc.sync.dma_start(out=dbg_d[nm].ap(), in_=tf)
                    nc.sync.dma_start(out=dbg_d["d_sd"].ap(), in_=sd)


            sts = []
            for rep in range(reps):
                for bi in range(BPC):
                    sts.append(emit_fcomp(emit_dma(bi)))
                    if len(sts) >= 2:
                        emit_att(emit_mid(sts.pop(0)))
            while sts:
                emit_att(emit_mid(sts.pop(0)))
            flush_out()

    _split_excess_waits(nc)
    return nc


def _pack_weights(inputs):
    """Host-side bf16 weight packing."""
    Wq = np.asarray(inputs["Wq"], dtype=np.float32)
    Wkv = np.asarray(inputs["Wkv"], dtype=np.float32)
    proj_w = np.asarray(inputs["proj_w"], dtype=np.float32)
    proj_b = np.asarray(inputs["proj_b"], dtype=np.float32)

    def pack_t(w, nkt, ncol):
        # w: [c_in, c_out] -> [128, nkt, ncol]
        return np.ascontiguousarray(
            w.reshape(nkt, 128, ncol).transpose(1, 0, 2))

    wq_t = Wq.T  # [512 in, 512 out]
    wkv_t = Wkv.T[_PERM, :]  # [1024 in(perm), 1024 out]
    proj_t = proj_w.T

    wqp = pack_t(wq_t, 4, 512)
    wk2p = pack_t(wkv_t[0:512], 4, 1024)
    wk1p = pack_t(wkv_t[512:1024], 4, 1024)
    wpp = pack_t(proj_t, 4, 512)

    ah = _pool_mats()
    wsm = np.zeros((128, WS), dtype=np.float32)
    wsm[:, _C_AH1:_C_AH1 + 16] = ah[0]
    wsm[:, _C_AH2:_C_AH2 + 32] = ah[1]
    wsm[:, _C_AH3:_C_AH3 + 64] = ah[2]
    g4 = np.asarray(inputs["ln4_g"], dtype=np.float32)
    b4 = np.asarray(inputs["ln4_b"], dtype=np.float32)
    wsm[:, _C_G4:_C_G4 + 4] = g4.reshape(4, 128).T
    wsm[:, _C_B4:_C_B4 + 4] = b4.reshape(4, 128).T
    for i in range(3):
        g = np.asarray(inputs[f"ln{i+1}_g"], dtype=np.float32)
        b = np.asarray(inputs[f"ln{i+1}_b"], dtype=np.float32)
        if i == 0:
            wsm[0:64, _C_GB] = g
            wsm[0:64, _C_BB] = b
        elif i == 1:
            wsm[:, _C_GB + 1] = g
            wsm[:, _C_BB + 1] = b
        else:
            wsm[:, _C_GB + 2] = g[0:128]
            wsm[:, _C_GB + 3] = g[128:256]
            wsm[0:64, _C_GB + 4] = g[256:320]
            wsm[:, _C_BB + 2] = b[0:128]
            wsm[:, _C_BB + 3] = b[128:256]
            wsm[0:64, _C_BB + 4] = b[256:320]
    for i, cb in enumerate((64, 128, 320, 512)):
        wsm[:, _C_INVCB + i] = 1.0 / cb
    wsm[:, _C_M4] = -1.0 / 512
    wsm[0, _R_EPS512:_R_EPS512 + 512] = EPS
    wsm[0, _R_PJB:_R_PJB + 512] = proj_b

    srwall = np.zeros((128, 1152), dtype=np.float32)
    srb_rows = {}
    for i, (c, off, roff) in enumerate(
            [(64, 0, _R_SRB1), (128, 64, _R_SRB2), (320, 192, _R_SRB3)]):
        w = np.asarray(inputs[f"sr{i+1}_w"], dtype=np.float32)  # [c_out, c_in]
        bsr = np.asarray(inputs[f"sr{i+1}_b"], dtype=np.float32)
        wt = w.T  # [c_in, c_out]
        wt = wt - wt.mean(axis=1, keepdims=True)  # fold LN mean-centering
        bsr = bsr - bsr.mean()
        if c == 64:
            srwall[0:64, 0:64] = wt
        elif c == 128:
            srwall[:, 64:192] = wt
        else:
            # [320, 320] -> [128, 3, 320] kt-blocks (kt2 only 64 rows)
            blk = np.zeros((128, 3, 320), dtype=np.float32)
            blk[:, 0] = wt[0:128]
            blk[:, 1] = wt[128:256]
            blk[0:64, 2] = wt[256:320]
            srwall[:, 192:1152] = blk.reshape(128, 960)
        wsm[0, roff:roff + c] = bsr
        if c == 64:
            wsm[0:64, _C_SRBC] = bsr
        elif c == 128:
            wsm[:, _C_SRBC + 1] = bsr
        else:
            wsm[:, _C_SRBC + 2] = bsr[0:128]
            wsm[:, _C_SRBC + 3] = bsr[128:256]
            wsm[0:64, _C_SRBC + 4] = bsr[256:320]
        srb_rows[i] = bsr

    import ml_dtypes

    def cast(a):
        return np.ascontiguousarray(a.astype(ml_dtypes.bfloat16))

    return {
        "wsmall": cast(wsm),
        "srwall": cast(srwall),
        "wqp": cast(wqp),
        "wk1p": cast(wk1p),
        "wk2p": cast(wk2p),
        "wpp": cast(wpp),
    }


def kernel(**inputs):
    x = np.ascontiguousarray(inputs["x"], dtype=np.float32)
    y1 = np.ascontiguousarray(inputs["y1"], dtype=np.float32)
    y2 = np.ascontiguousarray(inputs["y2"], dtype=np.float32)
    y3 = np.ascontiguousarray(inputs["y3"], dtype=np.float32)

    common = _pack_weights(inputs)

    nc = build_module()
    in_maps = []
    for c in range(NCORES):
        sl = slice(c * BPC, (c + 1) * BPC)
        m = dict(common)
        m["x"] = x[sl]
        m["y1"] = y1[sl]
        m["y2"] = y2[sl]
        m["y3"] = y3[sl]
        in_maps.append(m)

    res = run_bass_kernel_spmd(nc, in_maps, core_ids=list(range(NCORES)))
    return np.concatenate([r["out"] for r in res.results], axis=0)


if __name__ == "__main__":
    pass
